# revision 25
# baseline (speedup 1.0000x reference)
"""Trainium2 Bass kernel for BasicNonLocalBlock (N=4, C=512, H=W=64, KC=VC=256, OC=512).

Sharding: 8 cores = 4 images x 2 query-halves. Each core holds one full image
(for K/V) and computes attention output for 2048 of the 4096 queries.

Per-core dataflow (all matmuls float32r, free dim 512 or 256):
  K  [256,4096] = fold(Wk,BN) @ x            (kc on partitions)
  Q  [256,2048] = fold(Wq,BN,1/sqrt(KC)) @ x_qhalf
  V^T[4096,256] = x^T @ WvT + bv             (kpos on partitions; no transposes)
  per 512-query block, streamed over 32 key chunks of 128:
    S^T[128,512] = K_chunk^T Q_block   (PSUM)
    P^T = exp(S^T)                     (scalar engine, PSUM->SBUF)
    sums[1,512]  += ones^T P^T         (PE accumulation)
    ctx[vc,512]  += V^T_chunk^T P^T    (PE accumulation, vc in 2 chunks)
  r = 1/sums  (transposed to [128,4] via DRAM bounce)
  out^T[q,oc] = ctx^T @ WWT, scaled by r (per-partition) + bW broadcast
Host folds BN + attention scale into weights, transposes/assembles output.
"""

import sys
import types
from contextlib import ExitStack

import numpy as np

# ---------------------------------------------------------------------------
# Environment shims (axon image lacks antenv.axon_hooks; walrus rejects >2
# sync waits on the tail Drain emitted by TileContext).
# ---------------------------------------------------------------------------


def _install_ntff_hook_shim():
    try:
        import antenv
    except ImportError:
        return
    if "antenv.axon_hooks" in sys.modules:
        return
    mod = types.ModuleType("antenv.axon_hooks")
    mod._hook = None

    def set_axon_ntff_profile_hook(h):
        mod._hook = h

    def get_axon_ntff_profile_hook():
        return mod._hook

    mod.set_axon_ntff_profile_hook = set_axon_ntff_profile_hook
    mod.get_axon_ntff_profile_hook = get_axon_ntff_profile_hook
    sys.modules["antenv.axon_hooks"] = mod
    antenv.axon_hooks = mod
    try:
        if "/root/.axon_site" not in sys.path:
            sys.path.insert(0, "/root/.axon_site")
        from trn_agent_boot.trn_boot import _ntff_profile_via_ctypes

        hook = _ntff_profile_via_ctypes("/opt/axon/libaxon_pjrt.so")
        if hook is not None:
            mod._hook = hook
    except Exception:
        pass


_install_ntff_hook_shim()

import concourse.bass as bass
import concourse.bass_isa as bass_isa
import concourse.tile as tile
from concourse import mybir
from concourse.bass_utils import run_bass_kernel_spmd
from concourse.vector_clock import ScopedClock

F32 = mybir.dt.float32
F32R = mybir.dt.float32r
ACT = mybir.ActivationFunctionType


def _patched_drain_and_barrier(self, tick_clock, wait_clock):
    nc = self.nc
    probe = nc.sync.nop(nofuse=True, hint="drain_waits_probe")
    wait_clock.add_sem_waits(probe.ins, ScopedClock({None: tick_clock.global_clock}))
    si = probe.ins.sync_info
    waits = list(si.on_wait or []) if si is not None else []
    if si is not None:
        si.on_wait = waits[:1]
    for w in waits[1:]:
        n = nc.sync.nop(nofuse=True, hint="drain_waits_extra")
        n.ins.sync_info = mybir.SyncInfo(on_wait=[w], on_update=[])
    nc.sync.drain()
    nc.all_engine_barrier()
    assert self.sems is not None
    popped = nc._tile_sem_poison_stack.pop()
    assert popped is self._sem_poison
    nc.clear_and_free_semaphores(list(self.sems.allocated().values()))


tile.TileContext._drain_and_barrier = _patched_drain_and_barrier


def _split_excess_waits(nc):
    """Walrus CoreV3 codegen limits embedded sync waits per instruction
    (1 for self-loading Matmult's LDWEIGHTS struct, 2 elsewhere). Move the
    excess onto same-engine NOPs inserted just before."""
    n_split = 0
    for fn in nc.m.functions:
        for blk in fn.blocks:
            new_insts = []
            for inst in blk.instructions:
                max_waits = 1
                si = getattr(inst, "sync_info", None)
                if si is not None and si.on_wait and len(si.on_wait) > max_waits:
                    waits = list(si.on_wait)
                    extra = waits[:-max_waits]
                    si.on_wait = waits[-max_waits:]
                    for i in range(0, len(extra), max_waits):
                        n_split += 1
                        nop = mybir.InstNoOp(
                            name=f"{inst.name}-ws{i}",
                            engine=inst.engine,
                            ins=[], outs=[],
                            sync_info=mybir.SyncInfo(
                                on_wait=extra[i:i + max_waits], on_update=[]),
                            bass_nofuse=True,
                        )
                        new_insts.append(nop)
                new_insts.append(inst)
            blk.instructions[:] = new_insts
    return n_split

# ---------------------------------------------------------------------------
# Problem constants (hardcoded; kernel.py must be self-contained)
# ---------------------------------------------------------------------------
N_IMG, C, H, W = 4, 512, 64, 64
KC, VC, OC = 256, 256, 512
L = H * W  # 4096
QH = L // 2  # queries per core
N_CORES = 8
EPS = 1e-5

NCC = C // 128  # 4   c chunks
NKC = KC // 128  # 2  kc chunks
NVC = VC // 128  # 2  vc chunks
NKI = L // 128  # 32  key chunks
NQB = QH // 512  # 4  query blocks per core
NLB = L // 512  # 8   l blocks


def _bcast(ap, p=128):
    """Broadcast a 1-D DRAM AP across p partitions."""
    return bass.AP(tensor=ap.tensor, offset=ap.offset, ap=[[0, p], list(ap.ap[0])])


def _build_program():
    nc = bass.Bass("TRN2", target_bir_lowering=False, debug=False,
                   num_devices=N_CORES)

    xq_ap = nc.dram_tensor("xq", [C, QH], F32R, kind="ExternalInput").ap()
    xo_ap = nc.dram_tensor("xo", [C, QH], F32R, kind="ExternalInput").ap()
    wqT_ap = nc.dram_tensor("wqT", [C, KC], F32R, kind="ExternalInput").ap()
    wkT_ap = nc.dram_tensor("wkT", [C, KC], F32R, kind="ExternalInput").ap()
    wvT_ap = nc.dram_tensor("wvT", [C, VC], F32R, kind="ExternalInput").ap()
    wWT_ap = nc.dram_tensor("wWT", [VC, OC], F32R, kind="ExternalInput").ap()
    bq_ap = nc.dram_tensor("bq", [KC], F32, kind="ExternalInput").ap()
    bk_ap = nc.dram_tensor("bk", [KC], F32, kind="ExternalInput").ap()
    bv_ap = nc.dram_tensor("bv", [VC], F32, kind="ExternalInput").ap()
    bW_ap = nc.dram_tensor("bW", [OC], F32, kind="ExternalInput").ap()
    out_ap = nc.dram_tensor("out_t", [QH, OC], F32, kind="ExternalOutput").ap()

    NQT = QH // 512  # 4 column-quarters per half

    with tile.TileContext(nc) as tc, ExitStack() as stack:
        consts = stack.enter_context(tc.tile_pool(name="consts", bufs=1))
        persist = stack.enter_context(tc.tile_pool(name="persist", bufs=1))
        # one shared pool for ALL transient matmul PSUM outputs (2 banks),
        # accumulators get their own pools: sum+rtp 2, ctx 2, out 2 -> 8 total
        mm_ps = stack.enter_context(tc.tile_pool(name="mm_ps", bufs=3,
                                                 space="PSUM"))
        ctx_psum = stack.enter_context(tc.tile_pool(name="ctx_psum", bufs=1,
                                                    space="PSUM"))
        o_psum = stack.enter_context(tc.tile_pool(name="o_psum", bufs=3,
                                                  space="PSUM"))
        acc_pool = stack.enter_context(tc.tile_pool(name="acc_sb", bufs=2))
        pt_pool = stack.enter_context(tc.tile_pool(name="pt", bufs=8))
        ctx_pool = stack.enter_context(tc.tile_pool(name="ctx_sb", bufs=2))
        o_pool = stack.enter_context(tc.tile_pool(name="o_sb", bufs=2))
        r_pool = stack.enter_context(tc.tile_pool(name="r_sb", bufs=2))
        dram_pool = stack.enter_context(tc.tile_pool(name="dramp", bufs=2,
                                                     space="DRAM"))
        xo_pool = stack.enter_context(tc.tile_pool(name="xopool", bufs=1))

        # ---- weights / consts ----
        wq_s = consts.tile([128, NCC, KC], F32R, tag="wq")
        nc.sync.dma_start(wq_s[:], wqT_ap.rearrange("(a p) k -> p a k", p=128))
        wk_s = consts.tile([128, NCC, KC], F32R, tag="wk")
        nc.sync.dma_start(wk_s[:], wkT_ap.rearrange("(a p) k -> p a k", p=128))
        wv_s = consts.tile([128, NCC, VC], F32R, tag="wv")
        nc.sync.dma_start(wv_s[:], wvT_ap.rearrange("(a p) k -> p a k", p=128))
        wW_s = consts.tile([128, NVC, OC], F32R, tag="wW")
        nc.sync.dma_start(wW_s[:], wWT_ap.rearrange("(a p) k -> p a k", p=128))
        bq_s = consts.tile([128, NKC], F32, tag="bq")
        nc.sync.dma_start(bq_s[:], bq_ap.rearrange("(a p) -> p a", p=128))
        bk_s = consts.tile([128, NKC], F32, tag="bk")
        nc.sync.dma_start(bk_s[:], bk_ap.rearrange("(a p) -> p a", p=128))
        bv_s = consts.tile([128, VC], F32, tag="bv")
        nc.sync.dma_start(bv_s[:], _bcast(bv_ap))
        bW_s = consts.tile([128, OC], F32, tag="bW")
        nc.sync.dma_start(bW_s[:], _bcast(bW_ap))
        ones_f = consts.tile([128, 1], F32, tag="onesf")
        nc.vector.memset(ones_f[:], 1.0)
        ones_s = consts.tile([128, 1], F32R, tag="ones")
        nc.vector.tensor_copy(ones_s[:], ones_f[:])
        warm_exp = consts.tile([128, 1], F32, tag="wexp")
        ones_mf = consts.tile([128, 128], F32, tag="onesmf")
        nc.vector.memset(ones_mf[:], 1.0)
        ones_m = consts.tile([128, 128], F32R, tag="onesm")
        nc.vector.tensor_copy(ones_m[:], ones_mf[:])

        # ---- persistent activations ----
        k_s = [persist.tile([128, L], F32R, tag=f"k{j}", name=f"k{j}")
               for j in range(NKC)]
        q_s = [persist.tile([128, QH], F32R, tag=f"q{j}", name=f"q{j}")
               for j in range(NKC)]
        vT_s = persist.tile([128, NKI, VC], F32R, tag="vT")

        def r(ap):
            return ap

        # ---- striped input DMAs (xq first; precise quarter-level deps) ----
        xq_s = [[None] * NQT for _ in range(NCC)]
        xo_s = [[None] * NQT for _ in range(NCC)]

        def stripe(pool, store, src_ap, pfx, t, ci):
            xt = pool.tile([128, 512], F32R, tag=f"{pfx}{ci}_{t}",
                           name=f"{pfx}{ci}_{t}")
            nc.sync.dma_start(
                xt[:], src_ap[ci * 128:(ci + 1) * 128, t * 512:(t + 1) * 512])
            store[ci][t] = xt

        def proj_quarter(xs, b, glob_b):
            # K chunk, (Q chunk if own half), V^T for one 512-column quarter
            for j in range(NKC):
                ps = mm_ps.tile([128, 512], F32, tag="mm", name=f"pk{j}_{glob_b}")
                for ci in range(NCC):
                    nc.tensor.matmul(
                        ps[:],
                        r(wk_s[:, ci, j * 128:(j + 1) * 128]),
                        r(xs[ci][b][:]),
                        start=(ci == 0), stop=(ci == NCC - 1))
                nc.vector.tensor_scalar_add(
                    k_s[j][:, glob_b * 512:(glob_b + 1) * 512], ps[:],
                    bk_s[:, j:j + 1])
            if glob_b < NQT:
                for j in range(NKC):
                    ps = mm_ps.tile([128, 512], F32, tag="mm",
                                    name=f"pq{j}_{glob_b}")
                    for ci in range(NCC):
                        nc.tensor.matmul(
                            ps[:],
                            r(wq_s[:, ci, j * 128:(j + 1) * 128]),
                            r(xs[ci][b][:]),
                            start=(ci == 0), stop=(ci == NCC - 1))
                    nc.vector.tensor_scalar_add(
                        q_s[j][:, glob_b * 512:(glob_b + 1) * 512], ps[:],
                        bq_s[:, j:j + 1])
            for kk in range(4 * b, 4 * b + 4):
                gki = glob_b * 4 + (kk - 4 * b)
                ps = mm_ps.tile([128, VC], F32, tag="mm", name=f"pv{gki}")
                for ci in range(NCC):
                    nc.tensor.matmul(
                        ps[:],
                        r(xs[ci][kk // 4][:, (kk % 4) * 128:(kk % 4 + 1) * 128]),
                        r(wv_s[:, ci, :]),
                        start=(ci == 0), stop=(ci == NCC - 1))
                nc.vector.tensor_add(vT_s[:, gki, :], ps[:], bv_s[:])

        # ---- attention ----
        def attn_qblock(qb, part, state):
            qo = qb * 512
            if part == "prefix":
                state.setdefault("pt", {})
            elif part in ("full", "first", "second") and "acc" not in state:
                state["acc"] = acc_pool.tile([128, 512], F32R, tag="acc",
                                             name=f"acc{qb}")
                state["ctx_ps"] = [
                    ctx_psum.tile([128, 512], F32, tag=f"ctx{j}",
                                  name=f"ctx{qb}_{j}")
                    for j in range(NVC)]
                state.setdefault("pt", {})
            acc = state.get("acc")
            ctx_ps = state.get("ctx_ps")
            pt_tiles = state["pt"]

            def emit_s(ki):
                ps = mm_ps.tile([128, 512], F32, tag="mm", name=f"s{qb}_{ki}")
                for j in range(NKC):
                    nc.tensor.matmul(
                        ps[:],
                        r(k_s[j][:, ki * 128:(ki + 1) * 128]),
                        r(q_s[j][:, qo:qo + 512]),
                        start=(j == 0), stop=(j == NKC - 1))
                pt = pt_pool.tile([128, 512], F32R, tag="pt",
                                  name=f"pt{qb}_{ki}")
                nc.scalar.activation(pt[:], ps[:], ACT.Exp)
                pt_tiles[ki] = pt

            if part == "prefix":
                for ki in range(6):
                    emit_s(ki)
                return

            def emit_acc(ki):
                pt = pt_tiles.pop(ki)
                if ki == 0:
                    nc.vector.tensor_copy(acc[:], pt[:])
                else:
                    nc.vector.tensor_add(acc[:], acc[:], pt[:])
                for j in range(NVC):
                    nc.tensor.matmul(
                        ctx_ps[j][:],
                        r(vT_s[:, ki, j * 128:(j + 1) * 128]),
                        r(pt[:]),
                        start=(ki == 0), stop=(ki == NKI - 1),
                        skip_group_check=True)

            kis = {"full": list(range(NKI)),
                   "first": list(range(NKI // 2)),
                   "second": list(range(NKI // 2, NKI))}[part]
            if kis[0] not in pt_tiles:
                emit_s(kis[0])
            last = kis[-1]
            for ki in kis:
                if ki < last and ki + 1 not in pt_tiles:
                    emit_s(ki + 1)
                emit_acc(ki)
            if part == "first":
                return
            if state.get("next") is not None:
                # pre-emit the next q-block's first S matmuls so the PE has
                # work while DVE copies ctx out of PSUM for this block
                nqb, nstate = state["next"]
                attn_qblock(nqb, "prefix", nstate)

            # softmax denominators: ones[128,128]^T @ acc broadcasts column
            # sums to every partition; per-128-column chains pipeline the
            # reduce -> reciprocal -> normalize -> output projection
            sb = mm_ps.tile([128, 512], F32, tag="mm", name=f"sbc{qb}")
            nc.tensor.matmul(sb[:], r(ones_m[:]), r(acc[:]),
                             start=True, stop=True, skip_group_check=True)
            for qs in range(4):
                cs = qs * 128
                rb = acc_pool.tile([128, 128], F32, tag="rbc", bufs=4,
                                   name=f"rbc{qb}_{qs}")
                nc.vector.reciprocal(rb[:], sb[:, cs:cs + 128])
                ctx_sb = []
                for j in range(NVC):
                    t = ctx_pool.tile([128, 128], F32R, tag=f"ctxs{j}",
                                      bufs=3, name=f"cs{qb}_{qs}_{j}")
                    nc.vector.tensor_mul(t[:], ctx_ps[j][:, cs:cs + 128],
                                         rb[:])
                    ctx_sb.append(t)
                ops = o_psum.tile([128, OC], F32, tag="ops", name=f"o{qb}_{qs}")
                for j in range(NVC):
                    nc.tensor.matmul(
                        ops[:], r(ctx_sb[j][:]), r(wW_s[:, j, :]),
                        start=(j == 0), stop=(j == NVC - 1))
                o_fin = o_pool.tile([128, OC], F32, tag="ofin",
                                    name=f"of{qb}_{qs}")
                nc.vector.tensor_add(o_fin[:], ops[:], bW_s[:])
                nc.sync.dma_start(
                    out_ap[qo + cs: qo + cs + 128, :], o_fin[:])

        # ---- program order ----
        with tc.tile_pool(name="xqpool", bufs=1) as xqp:
            for t in range(NQT):
                for ci in range(NCC):
                    stripe(xqp, xq_s, xq_ap, "xq", t, ci)
            for t in range(NQT):
                for ci in range(NCC):
                    stripe(xo_pool, xo_s, xo_ap, "xo", t, ci)

            # PE warm-up on the (tiny, early) weight tiles: release the HAM
            # clock throttle before the projections start
            for wi in range(24):
                wps = mm_ps.tile([1, KC], F32, tag="mm", name=f"warm{wi}")
                nc.tensor.matmul(wps[:], r(ones_s[:]), r(wq_s[:, 0, :]),
                                 start=True, stop=True, skip_group_check=True)

            for b in range(NQT):             # own half: K+Q+V^T per quarter
                proj_quarter(xq_s, b, b)
            # preload the Exp LUT (attention's first exp skips table load)
            nc.scalar.activation(warm_exp[:], ones_f[:], ACT.Exp)

            # qb0 own-half keys run while xo is still streaming in; the
            # other-half projections then fill the PE before qb0 resumes.
            states = [{} for _ in range(NQB)]
            for qb in range(NQB - 1):
                states[qb]["next"] = (qb + 1, states[qb + 1])
            states[NQB - 1]["next"] = None
            attn_qblock(0, "first", states[0])
            for b in range(NQT):
                proj_quarter(xo_s, b, NQT + b)
            attn_qblock(0, "second", states[0])
        for qb in range(1, NQB):
            attn_qblock(qb, "full", states[qb])

    _split_excess_waits(nc)
    return nc


_NC_CACHE = {}


def _get_nc():
    if "nc" not in _NC_CACHE:
        _NC_CACHE["nc"] = _build_program()
    return _NC_CACHE["nc"]


def _prep_in_maps(x, wq, bq, gq, betaq, mq, vq, wk, bk, gk, betak, mk, vk,
                  wv, bv, wW, bW):
    x = np.asarray(x, np.float32)
    invq = np.asarray(gq, np.float32) / np.sqrt(np.asarray(vq, np.float32) + EPS)
    invk = np.asarray(gk, np.float32) / np.sqrt(np.asarray(vk, np.float32) + EPS)
    scale = 1.0 / np.sqrt(np.float32(KC))
    wq_f = (np.asarray(wq, np.float32) * invq[:, None]) * scale
    bq_f = (np.asarray(bq, np.float32) * invq + np.asarray(betaq, np.float32)
            - np.asarray(mq, np.float32) * invq) * scale
    wk_f = np.asarray(wk, np.float32) * invk[:, None]
    bk_f = (np.asarray(bk, np.float32) * invk + np.asarray(betak, np.float32)
            - np.asarray(mk, np.float32) * invk)

    shared = {
        "wqT": np.ascontiguousarray(wq_f.T, np.float32),
        "wkT": np.ascontiguousarray(wk_f.T, np.float32),
        "wvT": np.ascontiguousarray(np.asarray(wv, np.float32).T, np.float32),
        "wWT": np.ascontiguousarray(np.asarray(wW, np.float32).T, np.float32),
        "bq": np.ascontiguousarray(bq_f, np.float32),
        "bk": np.ascontiguousarray(bk_f, np.float32),
        "bv": np.ascontiguousarray(np.asarray(bv, np.float32)),
        "bW": np.ascontiguousarray(np.asarray(bW, np.float32)),
    }
    in_maps = []
    for c in range(N_CORES):
        n, half = c // 2, c % 2
        x_img = x[n].reshape(C, L)
        xq = np.ascontiguousarray(x_img[:, half * QH:(half + 1) * QH])
        xo = np.ascontiguousarray(x_img[:, (1 - half) * QH:(2 - half) * QH])
        in_maps.append({"xq": xq, "xo": xo, **shared})
    return in_maps


def _assemble(results):
    full = np.empty((N_IMG, C if OC == C else OC, L), np.float32)
    for n in range(N_IMG):
        halves = [results[2 * n]["out_t"], results[2 * n + 1]["out_t"]]
        img = np.concatenate(halves, axis=0)  # [L, OC]
        full[n] = img.T
    return full.reshape(N_IMG, OC, H, W)


def run_bass(trace=False, **inputs):
    nc = _get_nc()
    in_maps = _prep_in_maps(**inputs)
    res = run_bass_kernel_spmd(nc, in_maps, core_ids=list(range(N_CORES)),
                               trace=trace)
    return _assemble(res.results), res


def kernel(**inputs):
    out, _ = run_bass(trace=False, **inputs)
    return out


# revision 26
# speedup vs baseline: 1.0163x; 1.0163x over previous
"""Trainium2 Bass kernel for BasicNonLocalBlock (N=4, C=512, H=W=64, KC=VC=256, OC=512).

Sharding: 8 cores = 4 images x 2 query-halves. Each core holds one full image
(for K/V) and computes attention output for 2048 of the 4096 queries.

Per-core dataflow (all matmuls float32r, free dim 512 or 256):
  K  [256,4096] = fold(Wk,BN) @ x            (kc on partitions)
  Q  [256,2048] = fold(Wq,BN,1/sqrt(KC)) @ x_qhalf
  V^T[4096,256] = x^T @ WvT + bv             (kpos on partitions; no transposes)
  per 512-query block, streamed over 32 key chunks of 128:
    S^T[128,512] = K_chunk^T Q_block   (PSUM)
    P^T = exp(S^T)                     (scalar engine, PSUM->SBUF)
    sums[1,512]  += ones^T P^T         (PE accumulation)
    ctx[vc,512]  += V^T_chunk^T P^T    (PE accumulation, vc in 2 chunks)
  r = 1/sums  (transposed to [128,4] via DRAM bounce)
  out^T[q,oc] = ctx^T @ WWT, scaled by r (per-partition) + bW broadcast
Host folds BN + attention scale into weights, transposes/assembles output.
"""

import sys
import types
from contextlib import ExitStack

import numpy as np

# ---------------------------------------------------------------------------
# Environment shims (axon image lacks antenv.axon_hooks; walrus rejects >2
# sync waits on the tail Drain emitted by TileContext).
# ---------------------------------------------------------------------------


def _install_ntff_hook_shim():
    try:
        import antenv
    except ImportError:
        return
    if "antenv.axon_hooks" in sys.modules:
        return
    mod = types.ModuleType("antenv.axon_hooks")
    mod._hook = None

    def set_axon_ntff_profile_hook(h):
        mod._hook = h

    def get_axon_ntff_profile_hook():
        return mod._hook

    mod.set_axon_ntff_profile_hook = set_axon_ntff_profile_hook
    mod.get_axon_ntff_profile_hook = get_axon_ntff_profile_hook
    sys.modules["antenv.axon_hooks"] = mod
    antenv.axon_hooks = mod
    try:
        if "/root/.axon_site" not in sys.path:
            sys.path.insert(0, "/root/.axon_site")
        from trn_agent_boot.trn_boot import _ntff_profile_via_ctypes

        hook = _ntff_profile_via_ctypes("/opt/axon/libaxon_pjrt.so")
        if hook is not None:
            mod._hook = hook
    except Exception:
        pass


_install_ntff_hook_shim()

import concourse.bass as bass
import concourse.bass_isa as bass_isa
import concourse.tile as tile
from concourse import mybir
from concourse.bass_utils import run_bass_kernel_spmd
from concourse.vector_clock import ScopedClock

F32 = mybir.dt.float32
F32R = mybir.dt.float32r
ACT = mybir.ActivationFunctionType


def _patched_drain_and_barrier(self, tick_clock, wait_clock):
    nc = self.nc
    probe = nc.sync.nop(nofuse=True, hint="drain_waits_probe")
    wait_clock.add_sem_waits(probe.ins, ScopedClock({None: tick_clock.global_clock}))
    si = probe.ins.sync_info
    waits = list(si.on_wait or []) if si is not None else []
    if si is not None:
        si.on_wait = waits[:1]
    for w in waits[1:]:
        n = nc.sync.nop(nofuse=True, hint="drain_waits_extra")
        n.ins.sync_info = mybir.SyncInfo(on_wait=[w], on_update=[])
    nc.sync.drain()
    nc.all_engine_barrier()
    assert self.sems is not None
    popped = nc._tile_sem_poison_stack.pop()
    assert popped is self._sem_poison
    nc.clear_and_free_semaphores(list(self.sems.allocated().values()))


tile.TileContext._drain_and_barrier = _patched_drain_and_barrier


def _split_excess_waits(nc):
    """Walrus CoreV3 codegen limits embedded sync waits per instruction
    (1 for self-loading Matmult's LDWEIGHTS struct, 2 elsewhere). Move the
    excess onto same-engine NOPs inserted just before."""
    n_split = 0
    for fn in nc.m.functions:
        for blk in fn.blocks:
            new_insts = []
            for inst in blk.instructions:
                max_waits = 1
                si = getattr(inst, "sync_info", None)
                if si is not None and si.on_wait and len(si.on_wait) > max_waits:
                    waits = list(si.on_wait)
                    extra = waits[:-max_waits]
                    si.on_wait = waits[-max_waits:]
                    for i in range(0, len(extra), max_waits):
                        n_split += 1
                        nop = mybir.InstNoOp(
                            name=f"{inst.name}-ws{i}",
                            engine=inst.engine,
                            ins=[], outs=[],
                            sync_info=mybir.SyncInfo(
                                on_wait=extra[i:i + max_waits], on_update=[]),
                            bass_nofuse=True,
                        )
                        new_insts.append(nop)
                new_insts.append(inst)
            blk.instructions[:] = new_insts
    return n_split

# ---------------------------------------------------------------------------
# Problem constants (hardcoded; kernel.py must be self-contained)
# ---------------------------------------------------------------------------
N_IMG, C, H, W = 4, 512, 64, 64
KC, VC, OC = 256, 256, 512
L = H * W  # 4096
QH = L // 2  # queries per core
N_CORES = 8
EPS = 1e-5

NCC = C // 128  # 4   c chunks
NKC = KC // 128  # 2  kc chunks
NVC = VC // 128  # 2  vc chunks
NKI = L // 128  # 32  key chunks
NQB = QH // 512  # 4  query blocks per core
NLB = L // 512  # 8   l blocks


def _bcast(ap, p=128):
    """Broadcast a 1-D DRAM AP across p partitions."""
    return bass.AP(tensor=ap.tensor, offset=ap.offset, ap=[[0, p], list(ap.ap[0])])


def _build_program():
    nc = bass.Bass("TRN2", target_bir_lowering=False, debug=False,
                   num_devices=N_CORES)

    xq_ap = nc.dram_tensor("xq", [C, QH], F32R, kind="ExternalInput").ap()
    xo_ap = nc.dram_tensor("xo", [C, QH], F32R, kind="ExternalInput").ap()
    wqT_ap = nc.dram_tensor("wqT", [C, KC], F32R, kind="ExternalInput").ap()
    wkT_ap = nc.dram_tensor("wkT", [C, KC], F32R, kind="ExternalInput").ap()
    wvT_ap = nc.dram_tensor("wvT", [C, VC], F32R, kind="ExternalInput").ap()
    wWT_ap = nc.dram_tensor("wWT", [VC, OC], F32R, kind="ExternalInput").ap()
    bq_ap = nc.dram_tensor("bq", [KC], F32, kind="ExternalInput").ap()
    bk_ap = nc.dram_tensor("bk", [KC], F32, kind="ExternalInput").ap()
    bv_ap = nc.dram_tensor("bv", [VC], F32, kind="ExternalInput").ap()
    bW_ap = nc.dram_tensor("bW", [OC], F32, kind="ExternalInput").ap()
    out_ap = nc.dram_tensor("out_t", [QH, OC], F32, kind="ExternalOutput").ap()

    NQT = QH // 512  # 4 column-quarters per half

    with tile.TileContext(nc) as tc, ExitStack() as stack:
        consts = stack.enter_context(tc.tile_pool(name="consts", bufs=1))
        persist = stack.enter_context(tc.tile_pool(name="persist", bufs=1))
        # one shared pool for ALL transient matmul PSUM outputs (2 banks),
        # accumulators get their own pools: sum+rtp 2, ctx 2, out 2 -> 8 total
        mm_ps = stack.enter_context(tc.tile_pool(name="mm_ps", bufs=3,
                                                 space="PSUM"))
        ctx_psum = stack.enter_context(tc.tile_pool(name="ctx_psum", bufs=1,
                                                    space="PSUM"))
        o_psum = stack.enter_context(tc.tile_pool(name="o_psum", bufs=2,
                                                  space="PSUM"))
        sb_psum = stack.enter_context(tc.tile_pool(name="sb_psum", bufs=1,
                                                   space="PSUM"))
        acc_pool = stack.enter_context(tc.tile_pool(name="acc_sb", bufs=2))
        pt_pool = stack.enter_context(tc.tile_pool(name="pt", bufs=8))
        ctx_pool = stack.enter_context(tc.tile_pool(name="ctx_sb", bufs=2))
        o_pool = stack.enter_context(tc.tile_pool(name="o_sb", bufs=2))
        r_pool = stack.enter_context(tc.tile_pool(name="r_sb", bufs=2))
        dram_pool = stack.enter_context(tc.tile_pool(name="dramp", bufs=2,
                                                     space="DRAM"))
        xo_pool = stack.enter_context(tc.tile_pool(name="xopool", bufs=1))

        # ---- weights / consts ----
        wq_s = consts.tile([128, NCC, KC], F32R, tag="wq")
        nc.sync.dma_start(wq_s[:], wqT_ap.rearrange("(a p) k -> p a k", p=128))
        wk_s = consts.tile([128, NCC, KC], F32R, tag="wk")
        nc.sync.dma_start(wk_s[:], wkT_ap.rearrange("(a p) k -> p a k", p=128))
        wv_s = consts.tile([128, NCC, VC], F32R, tag="wv")
        nc.sync.dma_start(wv_s[:], wvT_ap.rearrange("(a p) k -> p a k", p=128))
        wW_s = consts.tile([128, NVC, OC], F32R, tag="wW")
        nc.sync.dma_start(wW_s[:], wWT_ap.rearrange("(a p) k -> p a k", p=128))
        bq_s = consts.tile([128, NKC], F32, tag="bq")
        nc.sync.dma_start(bq_s[:], bq_ap.rearrange("(a p) -> p a", p=128))
        bk_s = consts.tile([128, NKC], F32, tag="bk")
        nc.sync.dma_start(bk_s[:], bk_ap.rearrange("(a p) -> p a", p=128))
        bv_s = consts.tile([128, VC], F32, tag="bv")
        nc.sync.dma_start(bv_s[:], _bcast(bv_ap))
        bW_s = consts.tile([128, OC], F32, tag="bW")
        nc.sync.dma_start(bW_s[:], _bcast(bW_ap))
        ones_f = consts.tile([128, 1], F32, tag="onesf")
        nc.vector.memset(ones_f[:], 1.0)
        ones_s = consts.tile([128, 1], F32R, tag="ones")
        nc.vector.tensor_copy(ones_s[:], ones_f[:])
        warm_exp = consts.tile([128, 1], F32, tag="wexp")
        ones_mf = consts.tile([128, 128], F32, tag="onesmf")
        nc.vector.memset(ones_mf[:], 1.0)
        ones_m = consts.tile([128, 128], F32R, tag="onesm")
        nc.vector.tensor_copy(ones_m[:], ones_mf[:])

        # ---- persistent activations ----
        k_s = [persist.tile([128, L], F32R, tag=f"k{j}", name=f"k{j}")
               for j in range(NKC)]
        q_s = [persist.tile([128, QH], F32R, tag=f"q{j}", name=f"q{j}")
               for j in range(NKC)]
        vT_s = persist.tile([128, NKI, VC], F32R, tag="vT")

        def r(ap):
            return ap

        # ---- striped input DMAs (xq first; precise quarter-level deps) ----
        xq_s = [[None] * NQT for _ in range(NCC)]
        xo_s = [[None] * NQT for _ in range(NCC)]

        def stripe(pool, store, src_ap, pfx, t, ci):
            xt = pool.tile([128, 512], F32R, tag=f"{pfx}{ci}_{t}",
                           name=f"{pfx}{ci}_{t}")
            nc.sync.dma_start(
                xt[:], src_ap[ci * 128:(ci + 1) * 128, t * 512:(t + 1) * 512])
            store[ci][t] = xt

        def proj_quarter(xs, b, glob_b):
            # K chunk, (Q chunk if own half), V^T for one 512-column quarter
            for j in range(NKC):
                ps = mm_ps.tile([128, 512], F32, tag="mm", name=f"pk{j}_{glob_b}")
                for ci in range(NCC):
                    nc.tensor.matmul(
                        ps[:],
                        r(wk_s[:, ci, j * 128:(j + 1) * 128]),
                        r(xs[ci][b][:]),
                        start=(ci == 0), stop=(ci == NCC - 1))
                nc.vector.tensor_scalar_add(
                    k_s[j][:, glob_b * 512:(glob_b + 1) * 512], ps[:],
                    bk_s[:, j:j + 1])
            if glob_b < NQT:
                for j in range(NKC):
                    ps = mm_ps.tile([128, 512], F32, tag="mm",
                                    name=f"pq{j}_{glob_b}")
                    for ci in range(NCC):
                        nc.tensor.matmul(
                            ps[:],
                            r(wq_s[:, ci, j * 128:(j + 1) * 128]),
                            r(xs[ci][b][:]),
                            start=(ci == 0), stop=(ci == NCC - 1))
                    nc.vector.tensor_scalar_add(
                        q_s[j][:, glob_b * 512:(glob_b + 1) * 512], ps[:],
                        bq_s[:, j:j + 1])
            for kk in range(4 * b, 4 * b + 4):
                gki = glob_b * 4 + (kk - 4 * b)
                ps = mm_ps.tile([128, VC], F32, tag="mm", name=f"pv{gki}")
                for ci in range(NCC):
                    nc.tensor.matmul(
                        ps[:],
                        r(xs[ci][kk // 4][:, (kk % 4) * 128:(kk % 4 + 1) * 128]),
                        r(wv_s[:, ci, :]),
                        start=(ci == 0), stop=(ci == NCC - 1))
                nc.vector.tensor_add(vT_s[:, gki, :], ps[:], bv_s[:])

        # ---- attention ----
        def attn_qblock(qb, part, state):
            qo = qb * 512
            if part == "prefix":
                state.setdefault("pt", {})
            elif part in ("full", "first", "second") and "acc" not in state:
                state["acc"] = acc_pool.tile([128, 512], F32R, tag="acc",
                                             name=f"acc{qb}")
                state["ctx_ps"] = [
                    ctx_psum.tile([128, 512], F32, tag=f"ctx{j}",
                                  name=f"ctx{qb}_{j}")
                    for j in range(NVC)]
                state.setdefault("pt", {})
            acc = state.get("acc")
            ctx_ps = state.get("ctx_ps")
            pt_tiles = state["pt"]

            def emit_s(ki):
                ps = mm_ps.tile([128, 512], F32, tag="mm", name=f"s{qb}_{ki}")
                for j in range(NKC):
                    nc.tensor.matmul(
                        ps[:],
                        r(k_s[j][:, ki * 128:(ki + 1) * 128]),
                        r(q_s[j][:, qo:qo + 512]),
                        start=(j == 0), stop=(j == NKC - 1))
                pt = pt_pool.tile([128, 512], F32R, tag="pt",
                                  name=f"pt{qb}_{ki}")
                nc.scalar.activation(pt[:], ps[:], ACT.Exp)
                pt_tiles[ki] = pt

            if part == "prefix":
                for ki in range(6):
                    emit_s(ki)
                return

            def emit_acc(ki):
                pt = pt_tiles.pop(ki)
                if ki == 0:
                    nc.vector.tensor_copy(acc[:], pt[:])
                else:
                    nc.vector.tensor_add(acc[:], acc[:], pt[:])
                for j in range(NVC):
                    nc.tensor.matmul(
                        ctx_ps[j][:],
                        r(vT_s[:, ki, j * 128:(j + 1) * 128]),
                        r(pt[:]),
                        start=(ki == 0), stop=(ki == NKI - 1),
                        skip_group_check=True)

            kis = {"full": list(range(NKI)),
                   "first": list(range(NKI // 2)),
                   "second": list(range(NKI // 2, NKI))}[part]
            if kis[0] not in pt_tiles:
                emit_s(kis[0])
            last = kis[-1]
            for ki in kis:
                if ki < last and ki + 1 not in pt_tiles:
                    emit_s(ki + 1)
                emit_acc(ki)
            if part == "first":
                return
            if state.get("next") is not None:
                # pre-emit the next q-block's first S matmuls so the PE has
                # work while DVE copies ctx out of PSUM for this block
                nqb, nstate = state["next"]
                attn_qblock(nqb, "prefix", nstate)

            # softmax denominators: ones[128,128]^T @ acc broadcasts column
            # sums to every partition; per-128-column chains pipeline the
            # reduce -> reciprocal -> normalize -> output projection
            sb = sb_psum.tile([128, 512], F32, tag="sbc", name=f"sbc{qb}")
            nc.tensor.matmul(sb[:], r(ones_m[:]), r(acc[:]),
                             start=True, stop=True, skip_group_check=True)
            for qs in range(4):
                cs = qs * 128
                rb = acc_pool.tile([128, 128], F32, tag="rbc", bufs=4,
                                   name=f"rbc{qb}_{qs}")
                nc.vector.reciprocal(rb[:], sb[:, cs:cs + 128])
                ctx_sb = []
                for j in range(NVC):
                    t = ctx_pool.tile([128, 128], F32R, tag=f"ctxs{j}",
                                      bufs=3, name=f"cs{qb}_{qs}_{j}")
                    nc.vector.tensor_mul(t[:], ctx_ps[j][:, cs:cs + 128],
                                         rb[:])
                    ctx_sb.append(t)
                ops = o_psum.tile([128, OC], F32, tag="ops", name=f"o{qb}_{qs}")
                for j in range(NVC):
                    nc.tensor.matmul(
                        ops[:], r(ctx_sb[j][:]), r(wW_s[:, j, :]),
                        start=(j == 0), stop=(j == NVC - 1))
                o_fin = o_pool.tile([128, OC], F32, tag="ofin",
                                    name=f"of{qb}_{qs}")
                nc.vector.tensor_add(o_fin[:], ops[:], bW_s[:])
                nc.sync.dma_start(
                    out_ap[qo + cs: qo + cs + 128, :], o_fin[:])

        # ---- program order ----
        with tc.tile_pool(name="xqpool", bufs=1) as xqp:
            for t in range(NQT):
                for ci in range(NCC):
                    stripe(xqp, xq_s, xq_ap, "xq", t, ci)
            for t in range(NQT):
                for ci in range(NCC):
                    stripe(xo_pool, xo_s, xo_ap, "xo", t, ci)

            # PE warm-up on the (tiny, early) weight tiles: release the HAM
            # clock throttle before the projections start
            for wi in range(24):
                wps = mm_ps.tile([1, KC], F32, tag="mm", name=f"warm{wi}")
                nc.tensor.matmul(wps[:], r(ones_s[:]), r(wq_s[:, 0, :]),
                                 start=True, stop=True, skip_group_check=True)

            for b in range(NQT):             # own half: K+Q+V^T per quarter
                proj_quarter(xq_s, b, b)
            # preload the Exp LUT (attention's first exp skips table load)
            nc.scalar.activation(warm_exp[:], ones_f[:], ACT.Exp)

            # qb0 own-half keys run while xo is still streaming in; the
            # other-half projections then fill the PE before qb0 resumes.
            states = [{} for _ in range(NQB)]
            for qb in range(NQB - 1):
                states[qb]["next"] = (qb + 1, states[qb + 1])
            states[NQB - 1]["next"] = None
            attn_qblock(0, "first", states[0])
            for b in range(NQT):
                proj_quarter(xo_s, b, NQT + b)
            attn_qblock(0, "second", states[0])
        for qb in range(1, NQB):
            attn_qblock(qb, "full", states[qb])

    _split_excess_waits(nc)
    return nc


_NC_CACHE = {}


def _get_nc():
    if "nc" not in _NC_CACHE:
        _NC_CACHE["nc"] = _build_program()
    return _NC_CACHE["nc"]


def _prep_in_maps(x, wq, bq, gq, betaq, mq, vq, wk, bk, gk, betak, mk, vk,
                  wv, bv, wW, bW):
    x = np.asarray(x, np.float32)
    invq = np.asarray(gq, np.float32) / np.sqrt(np.asarray(vq, np.float32) + EPS)
    invk = np.asarray(gk, np.float32) / np.sqrt(np.asarray(vk, np.float32) + EPS)
    scale = 1.0 / np.sqrt(np.float32(KC))
    wq_f = (np.asarray(wq, np.float32) * invq[:, None]) * scale
    bq_f = (np.asarray(bq, np.float32) * invq + np.asarray(betaq, np.float32)
            - np.asarray(mq, np.float32) * invq) * scale
    wk_f = np.asarray(wk, np.float32) * invk[:, None]
    bk_f = (np.asarray(bk, np.float32) * invk + np.asarray(betak, np.float32)
            - np.asarray(mk, np.float32) * invk)

    shared = {
        "wqT": np.ascontiguousarray(wq_f.T, np.float32),
        "wkT": np.ascontiguousarray(wk_f.T, np.float32),
        "wvT": np.ascontiguousarray(np.asarray(wv, np.float32).T, np.float32),
        "wWT": np.ascontiguousarray(np.asarray(wW, np.float32).T, np.float32),
        "bq": np.ascontiguousarray(bq_f, np.float32),
        "bk": np.ascontiguousarray(bk_f, np.float32),
        "bv": np.ascontiguousarray(np.asarray(bv, np.float32)),
        "bW": np.ascontiguousarray(np.asarray(bW, np.float32)),
    }
    in_maps = []
    for c in range(N_CORES):
        n, half = c // 2, c % 2
        x_img = x[n].reshape(C, L)
        xq = np.ascontiguousarray(x_img[:, half * QH:(half + 1) * QH])
        xo = np.ascontiguousarray(x_img[:, (1 - half) * QH:(2 - half) * QH])
        in_maps.append({"xq": xq, "xo": xo, **shared})
    return in_maps


def _assemble(results):
    full = np.empty((N_IMG, C if OC == C else OC, L), np.float32)
    for n in range(N_IMG):
        halves = [results[2 * n]["out_t"], results[2 * n + 1]["out_t"]]
        img = np.concatenate(halves, axis=0)  # [L, OC]
        full[n] = img.T
    return full.reshape(N_IMG, OC, H, W)


def run_bass(trace=False, **inputs):
    nc = _get_nc()
    in_maps = _prep_in_maps(**inputs)
    res = run_bass_kernel_spmd(nc, in_maps, core_ids=list(range(N_CORES)),
                               trace=trace)
    return _assemble(res.results), res


def kernel(**inputs):
    out, _ = run_bass(trace=False, **inputs)
    return out


# revision 27
# speedup vs baseline: 1.0484x; 1.0317x over previous
"""Trainium2 Bass kernel for BasicNonLocalBlock (N=4, C=512, H=W=64, KC=VC=256, OC=512).

Sharding: 8 cores = 4 images x 2 query-halves. Each core holds one full image
(for K/V) and computes attention output for 2048 of the 4096 queries.

Per-core dataflow (all matmuls float32r, free dim 512 or 256):
  K  [256,4096] = fold(Wk,BN) @ x            (kc on partitions)
  Q  [256,2048] = fold(Wq,BN,1/sqrt(KC)) @ x_qhalf
  V^T[4096,256] = x^T @ WvT + bv             (kpos on partitions; no transposes)
  per 512-query block, streamed over 32 key chunks of 128:
    S^T[128,512] = K_chunk^T Q_block   (PSUM)
    P^T = exp(S^T)                     (scalar engine, PSUM->SBUF)
    sums[1,512]  += ones^T P^T         (PE accumulation)
    ctx[vc,512]  += V^T_chunk^T P^T    (PE accumulation, vc in 2 chunks)
  r = 1/sums  (transposed to [128,4] via DRAM bounce)
  out^T[q,oc] = ctx^T @ WWT, scaled by r (per-partition) + bW broadcast
Host folds BN + attention scale into weights, transposes/assembles output.
"""

import sys
import types
from contextlib import ExitStack

import numpy as np

# ---------------------------------------------------------------------------
# Environment shims (axon image lacks antenv.axon_hooks; walrus rejects >2
# sync waits on the tail Drain emitted by TileContext).
# ---------------------------------------------------------------------------


def _install_ntff_hook_shim():
    try:
        import antenv
    except ImportError:
        return
    if "antenv.axon_hooks" in sys.modules:
        return
    mod = types.ModuleType("antenv.axon_hooks")
    mod._hook = None

    def set_axon_ntff_profile_hook(h):
        mod._hook = h

    def get_axon_ntff_profile_hook():
        return mod._hook

    mod.set_axon_ntff_profile_hook = set_axon_ntff_profile_hook
    mod.get_axon_ntff_profile_hook = get_axon_ntff_profile_hook
    sys.modules["antenv.axon_hooks"] = mod
    antenv.axon_hooks = mod
    try:
        if "/root/.axon_site" not in sys.path:
            sys.path.insert(0, "/root/.axon_site")
        from trn_agent_boot.trn_boot import _ntff_profile_via_ctypes

        hook = _ntff_profile_via_ctypes("/opt/axon/libaxon_pjrt.so")
        if hook is not None:
            mod._hook = hook
    except Exception:
        pass


_install_ntff_hook_shim()

import concourse.bass as bass
import concourse.bass_isa as bass_isa
import concourse.tile as tile
from concourse import mybir
from concourse.bass_utils import run_bass_kernel_spmd
from concourse.vector_clock import ScopedClock

F32 = mybir.dt.float32
F32R = mybir.dt.float32r
ACT = mybir.ActivationFunctionType


def _patched_drain_and_barrier(self, tick_clock, wait_clock):
    nc = self.nc
    probe = nc.sync.nop(nofuse=True, hint="drain_waits_probe")
    wait_clock.add_sem_waits(probe.ins, ScopedClock({None: tick_clock.global_clock}))
    si = probe.ins.sync_info
    waits = list(si.on_wait or []) if si is not None else []
    if si is not None:
        si.on_wait = waits[:1]
    for w in waits[1:]:
        n = nc.sync.nop(nofuse=True, hint="drain_waits_extra")
        n.ins.sync_info = mybir.SyncInfo(on_wait=[w], on_update=[])
    nc.sync.drain()
    nc.all_engine_barrier()
    assert self.sems is not None
    popped = nc._tile_sem_poison_stack.pop()
    assert popped is self._sem_poison
    nc.clear_and_free_semaphores(list(self.sems.allocated().values()))


tile.TileContext._drain_and_barrier = _patched_drain_and_barrier


def _split_excess_waits(nc):
    """Walrus CoreV3 codegen limits embedded sync waits per instruction
    (1 for self-loading Matmult's LDWEIGHTS struct, 2 elsewhere). Move the
    excess onto same-engine NOPs inserted just before."""
    n_split = 0
    for fn in nc.m.functions:
        for blk in fn.blocks:
            new_insts = []
            for inst in blk.instructions:
                max_waits = 1
                si = getattr(inst, "sync_info", None)
                if si is not None and si.on_wait and len(si.on_wait) > max_waits:
                    waits = list(si.on_wait)
                    extra = waits[:-max_waits]
                    si.on_wait = waits[-max_waits:]
                    for i in range(0, len(extra), max_waits):
                        n_split += 1
                        nop = mybir.InstNoOp(
                            name=f"{inst.name}-ws{i}",
                            engine=inst.engine,
                            ins=[], outs=[],
                            sync_info=mybir.SyncInfo(
                                on_wait=extra[i:i + max_waits], on_update=[]),
                            bass_nofuse=True,
                        )
                        new_insts.append(nop)
                new_insts.append(inst)
            blk.instructions[:] = new_insts
    return n_split

# ---------------------------------------------------------------------------
# Problem constants (hardcoded; kernel.py must be self-contained)
# ---------------------------------------------------------------------------
N_IMG, C, H, W = 4, 512, 64, 64
KC, VC, OC = 256, 256, 512
L = H * W  # 4096
QH = L // 2  # queries per core
N_CORES = 8
EPS = 1e-5

NCC = C // 128  # 4   c chunks
NKC = KC // 128  # 2  kc chunks
NVC = VC // 128  # 2  vc chunks
NKI = L // 128  # 32  key chunks
NQB = QH // 512  # 4  query blocks per core
NLB = L // 512  # 8   l blocks


def _bcast(ap, p=128):
    """Broadcast a 1-D DRAM AP across p partitions."""
    return bass.AP(tensor=ap.tensor, offset=ap.offset, ap=[[0, p], list(ap.ap[0])])


def _build_program():
    nc = bass.Bass("TRN2", target_bir_lowering=False, debug=False,
                   num_devices=N_CORES)

    xq_ap = nc.dram_tensor("xq", [C, QH], F32R, kind="ExternalInput").ap()
    xo_ap = nc.dram_tensor("xo", [C, QH], F32R, kind="ExternalInput").ap()
    wqT_ap = nc.dram_tensor("wqT", [C, KC], F32R, kind="ExternalInput").ap()
    wkT_ap = nc.dram_tensor("wkT", [C, KC], F32R, kind="ExternalInput").ap()
    wvT_ap = nc.dram_tensor("wvT", [C, VC], F32R, kind="ExternalInput").ap()
    wWT_ap = nc.dram_tensor("wWT", [VC, OC], F32R, kind="ExternalInput").ap()
    bq_ap = nc.dram_tensor("bq", [KC], F32, kind="ExternalInput").ap()
    bk_ap = nc.dram_tensor("bk", [KC], F32, kind="ExternalInput").ap()
    bv_ap = nc.dram_tensor("bv", [VC], F32, kind="ExternalInput").ap()
    bW_ap = nc.dram_tensor("bW", [OC], F32, kind="ExternalInput").ap()
    out_ap = nc.dram_tensor("out_t", [QH, OC], F32, kind="ExternalOutput").ap()

    NQT = QH // 512  # 4 column-quarters per half

    with tile.TileContext(nc) as tc, ExitStack() as stack:
        consts = stack.enter_context(tc.tile_pool(name="consts", bufs=1))
        persist = stack.enter_context(tc.tile_pool(name="persist", bufs=1))
        # one shared pool for ALL transient matmul PSUM outputs (2 banks),
        # accumulators get their own pools: sum+rtp 2, ctx 2, out 2 -> 8 total
        mm_ps = stack.enter_context(tc.tile_pool(name="mm_ps", bufs=3,
                                                 space="PSUM"))
        ctx_psum = stack.enter_context(tc.tile_pool(name="ctx_psum", bufs=1,
                                                    space="PSUM"))
        o_psum = stack.enter_context(tc.tile_pool(name="o_psum", bufs=2,
                                                  space="PSUM"))
        sb_psum = stack.enter_context(tc.tile_pool(name="sb_psum", bufs=1,
                                                   space="PSUM"))
        acc_pool = stack.enter_context(tc.tile_pool(name="acc_sb", bufs=2))
        pt_pool = stack.enter_context(tc.tile_pool(name="pt", bufs=8))
        ctx_pool = stack.enter_context(tc.tile_pool(name="ctx_sb", bufs=2))
        o_pool = stack.enter_context(tc.tile_pool(name="o_sb", bufs=2))
        r_pool = stack.enter_context(tc.tile_pool(name="r_sb", bufs=2))
        dram_pool = stack.enter_context(tc.tile_pool(name="dramp", bufs=2,
                                                     space="DRAM"))
        xo_pool = stack.enter_context(tc.tile_pool(name="xopool", bufs=1))

        # ---- weights / consts ----
        wq_s = consts.tile([128, NCC, KC], F32R, tag="wq")
        nc.sync.dma_start(wq_s[:], wqT_ap.rearrange("(a p) k -> p a k", p=128))
        wk_s = consts.tile([128, NCC, KC], F32R, tag="wk")
        nc.sync.dma_start(wk_s[:], wkT_ap.rearrange("(a p) k -> p a k", p=128))
        wv_s = consts.tile([128, NCC, VC], F32R, tag="wv")
        nc.sync.dma_start(wv_s[:], wvT_ap.rearrange("(a p) k -> p a k", p=128))
        wW_s = consts.tile([128, NVC, OC], F32R, tag="wW")
        nc.sync.dma_start(wW_s[:], wWT_ap.rearrange("(a p) k -> p a k", p=128))
        bq_s = consts.tile([128, NKC], F32, tag="bq")
        nc.sync.dma_start(bq_s[:], bq_ap.rearrange("(a p) -> p a", p=128))
        bk_s = consts.tile([128, NKC], F32, tag="bk")
        nc.sync.dma_start(bk_s[:], bk_ap.rearrange("(a p) -> p a", p=128))
        bv_s = consts.tile([128, VC], F32, tag="bv")
        nc.sync.dma_start(bv_s[:], _bcast(bv_ap))
        bW_s = consts.tile([128, OC], F32, tag="bW")
        nc.sync.dma_start(bW_s[:], _bcast(bW_ap))
        ones_f = consts.tile([128, 1], F32, tag="onesf")
        nc.vector.memset(ones_f[:], 1.0)
        ones_s = consts.tile([128, 1], F32R, tag="ones")
        nc.vector.tensor_copy(ones_s[:], ones_f[:])
        warm_exp = consts.tile([128, 1], F32, tag="wexp")
        ones_mf = consts.tile([128, 128], F32, tag="onesmf")
        nc.vector.memset(ones_mf[:], 1.0)
        ones_m = consts.tile([128, 128], F32R, tag="onesm")
        nc.vector.tensor_copy(ones_m[:], ones_mf[:])

        # ---- persistent activations ----
        k_s = [persist.tile([128, L], F32R, tag=f"k{j}", name=f"k{j}")
               for j in range(NKC)]
        q_s = [persist.tile([128, QH], F32R, tag=f"q{j}", name=f"q{j}")
               for j in range(NKC)]
        vT_s = persist.tile([128, NKI, VC], F32R, tag="vT")

        def r(ap):
            return ap

        # ---- striped input DMAs (xq first; precise quarter-level deps) ----
        xq_s = [[None] * NQT for _ in range(NCC)]
        xo_s = [[None] * NQT for _ in range(NCC)]

        def stripe(pool, store, src_ap, pfx, t, ci):
            xt = pool.tile([128, 512], F32R, tag=f"{pfx}{ci}_{t}",
                           name=f"{pfx}{ci}_{t}")
            nc.sync.dma_start(
                xt[:], src_ap[ci * 128:(ci + 1) * 128, t * 512:(t + 1) * 512])
            store[ci][t] = xt

        def proj_quarter(xs, b, glob_b):
            # K chunk, (Q chunk if own half), V^T for one 512-column quarter
            for j in range(NKC):
                ps = mm_ps.tile([128, 512], F32, tag="mm", name=f"pk{j}_{glob_b}")
                for ci in range(NCC):
                    nc.tensor.matmul(
                        ps[:],
                        r(wk_s[:, ci, j * 128:(j + 1) * 128]),
                        r(xs[ci][b][:]),
                        start=(ci == 0), stop=(ci == NCC - 1))
                nc.vector.tensor_scalar_add(
                    k_s[j][:, glob_b * 512:(glob_b + 1) * 512], ps[:],
                    bk_s[:, j:j + 1])
            if glob_b < NQT:
                for j in range(NKC):
                    ps = mm_ps.tile([128, 512], F32, tag="mm",
                                    name=f"pq{j}_{glob_b}")
                    for ci in range(NCC):
                        nc.tensor.matmul(
                            ps[:],
                            r(wq_s[:, ci, j * 128:(j + 1) * 128]),
                            r(xs[ci][b][:]),
                            start=(ci == 0), stop=(ci == NCC - 1))
                    nc.vector.tensor_scalar_add(
                        q_s[j][:, glob_b * 512:(glob_b + 1) * 512], ps[:],
                        bq_s[:, j:j + 1])
            for kk in range(4 * b, 4 * b + 4):
                gki = glob_b * 4 + (kk - 4 * b)
                ps = mm_ps.tile([128, VC], F32, tag="mm", name=f"pv{gki}")
                for ci in range(NCC):
                    nc.tensor.matmul(
                        ps[:],
                        r(xs[ci][kk // 4][:, (kk % 4) * 128:(kk % 4 + 1) * 128]),
                        r(wv_s[:, ci, :]),
                        start=(ci == 0), stop=(ci == NCC - 1))
                nc.vector.tensor_add(vT_s[:, gki, :], ps[:], bv_s[:])

        # ---- attention ----
        def attn_qblock(qb, part, state):
            qo = qb * 512
            if part == "prefix":
                state.setdefault("pt", {})
            elif part in ("full", "first", "second") and "acc" not in state:
                state["acc"] = acc_pool.tile([128, 512], F32R, tag="acc",
                                             name=f"acc{qb}")
                state["ctx_ps"] = [
                    ctx_psum.tile([128, 512], F32, tag=f"ctx{j}",
                                  name=f"ctx{qb}_{j}")
                    for j in range(NVC)]
                state.setdefault("pt", {})
            acc = state.get("acc")
            ctx_ps = state.get("ctx_ps")
            pt_tiles = state["pt"]

            def emit_s(ki):
                ps = mm_ps.tile([128, 512], F32, tag="mm", name=f"s{qb}_{ki}")
                for j in range(NKC):
                    nc.tensor.matmul(
                        ps[:],
                        r(k_s[j][:, ki * 128:(ki + 1) * 128]),
                        r(q_s[j][:, qo:qo + 512]),
                        start=(j == 0), stop=(j == NKC - 1))
                pt = pt_pool.tile([128, 512], F32R, tag="pt",
                                  name=f"pt{qb}_{ki}")
                nc.scalar.activation(pt[:], ps[:], ACT.Exp)
                pt_tiles[ki] = pt

            if part == "prefix":
                for ki in range(6):
                    emit_s(ki)
                return

            def emit_acc(ki):
                pt = pt_tiles.pop(ki)
                if ki == 0:
                    nc.vector.tensor_copy(acc[:], pt[:])
                else:
                    nc.vector.tensor_add(acc[:], acc[:], pt[:])
                for j in range(NVC):
                    nc.tensor.matmul(
                        ctx_ps[j][:],
                        r(vT_s[:, ki, j * 128:(j + 1) * 128]),
                        r(pt[:]),
                        start=(ki == 0), stop=(ki == NKI - 1),
                        skip_group_check=True)

            kis = {"full": list(range(NKI)),
                   "first": list(range(NKI // 2)),
                   "second": list(range(NKI // 2, NKI))}[part]
            if kis[0] not in pt_tiles:
                emit_s(kis[0])
            last = kis[-1]
            for ki in kis:
                if ki < last and ki + 1 not in pt_tiles:
                    emit_s(ki + 1)
                emit_acc(ki)
            if part == "first":
                return
            if state.get("next") is not None:
                # pre-emit the next q-block's first S matmuls so the PE has
                # work while DVE copies ctx out of PSUM for this block
                nqb, nstate = state["next"]
                attn_qblock(nqb, "prefix", nstate)

            # softmax denominators: ones[128,128]^T @ acc broadcasts column
            # sums to every partition; per-128-column chains pipeline the
            # reduce -> reciprocal -> normalize -> output projection
            sb = sb_psum.tile([128, 512], F32, tag="sbc", name=f"sbc{qb}")
            nc.tensor.matmul(sb[:], r(ones_m[:]), r(acc[:]),
                             start=True, stop=True, skip_group_check=True)
            rb = acc_pool.tile([128, 512], F32, tag="rbc", name=f"rbc{qb}")
            nc.vector.reciprocal(rb[:], sb[:])
            ctx_sb = []
            for j in range(NVC):
                t = ctx_pool.tile([128, 512], F32R, tag=f"ctxs{j}",
                                  name=f"cs{qb}_{j}")
                nc.vector.tensor_mul(t[:], ctx_ps[j][:], rb[:])
                ctx_sb.append(t)
            for qs in range(4):
                ops = o_psum.tile([128, OC], F32, tag="ops", name=f"o{qb}_{qs}")
                for j in range(NVC):
                    nc.tensor.matmul(
                        ops[:],
                        r(ctx_sb[j][:, qs * 128:(qs + 1) * 128]),
                        r(wW_s[:, j, :]),
                        start=(j == 0), stop=(j == NVC - 1))
                o_fin = o_pool.tile([128, OC], F32, tag="ofin",
                                    name=f"of{qb}_{qs}")
                nc.vector.tensor_add(o_fin[:], ops[:], bW_s[:])
                nc.sync.dma_start(
                    out_ap[qo + qs * 128: qo + (qs + 1) * 128, :], o_fin[:])

        # ---- program order ----
        with tc.tile_pool(name="xqpool", bufs=1) as xqp:
            for t in range(NQT):
                for ci in range(NCC):
                    stripe(xqp, xq_s, xq_ap, "xq", t, ci)
            for t in range(NQT):
                for ci in range(NCC):
                    stripe(xo_pool, xo_s, xo_ap, "xo", t, ci)

            # PE warm-up on the (tiny, early) weight tiles: release the HAM
            # clock throttle before the projections start
            for wi in range(24):
                wps = mm_ps.tile([1, KC], F32, tag="mm", name=f"warm{wi}")
                nc.tensor.matmul(wps[:], r(ones_s[:]), r(wq_s[:, 0, :]),
                                 start=True, stop=True, skip_group_check=True)

            for b in range(NQT):             # own half: K+Q+V^T per quarter
                proj_quarter(xq_s, b, b)
            # preload the Exp LUT (attention's first exp skips table load)
            nc.scalar.activation(warm_exp[:], ones_f[:], ACT.Exp)

            # qb0 own-half keys run while xo is still streaming in; the
            # other-half projections then fill the PE before qb0 resumes.
            states = [{} for _ in range(NQB)]
            for qb in range(NQB - 1):
                states[qb]["next"] = (qb + 1, states[qb + 1])
            states[NQB - 1]["next"] = None
            attn_qblock(0, "first", states[0])
            for b in range(NQT):
                proj_quarter(xo_s, b, NQT + b)
            attn_qblock(0, "second", states[0])
        for qb in range(1, NQB):
            attn_qblock(qb, "full", states[qb])

    _split_excess_waits(nc)
    return nc


_NC_CACHE = {}


def _get_nc():
    if "nc" not in _NC_CACHE:
        _NC_CACHE["nc"] = _build_program()
    return _NC_CACHE["nc"]


def _prep_in_maps(x, wq, bq, gq, betaq, mq, vq, wk, bk, gk, betak, mk, vk,
                  wv, bv, wW, bW):
    x = np.asarray(x, np.float32)
    invq = np.asarray(gq, np.float32) / np.sqrt(np.asarray(vq, np.float32) + EPS)
    invk = np.asarray(gk, np.float32) / np.sqrt(np.asarray(vk, np.float32) + EPS)
    scale = 1.0 / np.sqrt(np.float32(KC))
    wq_f = (np.asarray(wq, np.float32) * invq[:, None]) * scale
    bq_f = (np.asarray(bq, np.float32) * invq + np.asarray(betaq, np.float32)
            - np.asarray(mq, np.float32) * invq) * scale
    wk_f = np.asarray(wk, np.float32) * invk[:, None]
    bk_f = (np.asarray(bk, np.float32) * invk + np.asarray(betak, np.float32)
            - np.asarray(mk, np.float32) * invk)

    shared = {
        "wqT": np.ascontiguousarray(wq_f.T, np.float32),
        "wkT": np.ascontiguousarray(wk_f.T, np.float32),
        "wvT": np.ascontiguousarray(np.asarray(wv, np.float32).T, np.float32),
        "wWT": np.ascontiguousarray(np.asarray(wW, np.float32).T, np.float32),
        "bq": np.ascontiguousarray(bq_f, np.float32),
        "bk": np.ascontiguousarray(bk_f, np.float32),
        "bv": np.ascontiguousarray(np.asarray(bv, np.float32)),
        "bW": np.ascontiguousarray(np.asarray(bW, np.float32)),
    }
    in_maps = []
    for c in range(N_CORES):
        n, half = c // 2, c % 2
        x_img = x[n].reshape(C, L)
        xq = np.ascontiguousarray(x_img[:, half * QH:(half + 1) * QH])
        xo = np.ascontiguousarray(x_img[:, (1 - half) * QH:(2 - half) * QH])
        in_maps.append({"xq": xq, "xo": xo, **shared})
    return in_maps


def _assemble(results):
    full = np.empty((N_IMG, C if OC == C else OC, L), np.float32)
    for n in range(N_IMG):
        halves = [results[2 * n]["out_t"], results[2 * n + 1]["out_t"]]
        img = np.concatenate(halves, axis=0)  # [L, OC]
        full[n] = img.T
    return full.reshape(N_IMG, OC, H, W)


def run_bass(trace=False, **inputs):
    nc = _get_nc()
    in_maps = _prep_in_maps(**inputs)
    res = run_bass_kernel_spmd(nc, in_maps, core_ids=list(range(N_CORES)),
                               trace=trace)
    return _assemble(res.results), res


def kernel(**inputs):
    out, _ = run_bass(trace=False, **inputs)
    return out


# revision 31
# speedup vs baseline: 1.1140x; 1.0625x over previous
"""Trainium2 Bass kernel for BasicNonLocalBlock (N=4, C=512, H=W=64, KC=VC=256, OC=512).

Sharding: 8 cores = 4 images x 2 query-halves. Each core holds one full image
(for K/V) and computes attention output for 2048 of the 4096 queries.

Per-core dataflow (all matmuls float32r, free dim 512 or 256):
  K  [256,4096] = fold(Wk,BN) @ x            (kc on partitions)
  Q  [256,2048] = fold(Wq,BN,1/sqrt(KC)) @ x_qhalf
  V^T[4096,256] = x^T @ WvT + bv             (kpos on partitions; no transposes)
  per 512-query block, streamed over 32 key chunks of 128:
    S^T[128,512] = K_chunk^T Q_block   (PSUM)
    P^T = exp(S^T)                     (scalar engine, PSUM->SBUF)
    sums[1,512]  += ones^T P^T         (PE accumulation)
    ctx[vc,512]  += V^T_chunk^T P^T    (PE accumulation, vc in 2 chunks)
  r = 1/sums  (transposed to [128,4] via DRAM bounce)
  out^T[q,oc] = ctx^T @ WWT, scaled by r (per-partition) + bW broadcast
Host folds BN + attention scale into weights, transposes/assembles output.
"""

import sys
import types
from contextlib import ExitStack

import numpy as np

# ---------------------------------------------------------------------------
# Environment shims (axon image lacks antenv.axon_hooks; walrus rejects >2
# sync waits on the tail Drain emitted by TileContext).
# ---------------------------------------------------------------------------


def _install_ntff_hook_shim():
    try:
        import antenv
    except ImportError:
        return
    if "antenv.axon_hooks" in sys.modules:
        return
    mod = types.ModuleType("antenv.axon_hooks")
    mod._hook = None

    def set_axon_ntff_profile_hook(h):
        mod._hook = h

    def get_axon_ntff_profile_hook():
        return mod._hook

    mod.set_axon_ntff_profile_hook = set_axon_ntff_profile_hook
    mod.get_axon_ntff_profile_hook = get_axon_ntff_profile_hook
    sys.modules["antenv.axon_hooks"] = mod
    antenv.axon_hooks = mod
    try:
        if "/root/.axon_site" not in sys.path:
            sys.path.insert(0, "/root/.axon_site")
        from trn_agent_boot.trn_boot import _ntff_profile_via_ctypes

        hook = _ntff_profile_via_ctypes("/opt/axon/libaxon_pjrt.so")
        if hook is not None:
            mod._hook = hook
    except Exception:
        pass


_install_ntff_hook_shim()

import concourse.bass as bass
import concourse.bass_isa as bass_isa
import concourse.tile as tile
from concourse import mybir
from concourse.bass_utils import run_bass_kernel_spmd
from concourse.vector_clock import ScopedClock

F32 = mybir.dt.float32
F32R = mybir.dt.float32r
ACT = mybir.ActivationFunctionType


def _patched_drain_and_barrier(self, tick_clock, wait_clock):
    nc = self.nc
    probe = nc.sync.nop(nofuse=True, hint="drain_waits_probe")
    wait_clock.add_sem_waits(probe.ins, ScopedClock({None: tick_clock.global_clock}))
    si = probe.ins.sync_info
    waits = list(si.on_wait or []) if si is not None else []
    if si is not None:
        si.on_wait = waits[:1]
    for w in waits[1:]:
        n = nc.sync.nop(nofuse=True, hint="drain_waits_extra")
        n.ins.sync_info = mybir.SyncInfo(on_wait=[w], on_update=[])
    nc.sync.drain()
    nc.all_engine_barrier()
    assert self.sems is not None
    popped = nc._tile_sem_poison_stack.pop()
    assert popped is self._sem_poison
    nc.clear_and_free_semaphores(list(self.sems.allocated().values()))


tile.TileContext._drain_and_barrier = _patched_drain_and_barrier


def _split_excess_waits(nc):
    """Walrus CoreV3 codegen limits embedded sync waits per instruction
    (1 for self-loading Matmult's LDWEIGHTS struct, 2 elsewhere). Move the
    excess onto same-engine NOPs inserted just before."""
    n_split = 0
    for fn in nc.m.functions:
        for blk in fn.blocks:
            new_insts = []
            for inst in blk.instructions:
                max_waits = 1
                si = getattr(inst, "sync_info", None)
                if si is not None and si.on_wait and len(si.on_wait) > max_waits:
                    waits = list(si.on_wait)
                    extra = waits[:-max_waits]
                    si.on_wait = waits[-max_waits:]
                    for i in range(0, len(extra), max_waits):
                        n_split += 1
                        nop = mybir.InstNoOp(
                            name=f"{inst.name}-ws{i}",
                            engine=inst.engine,
                            ins=[], outs=[],
                            sync_info=mybir.SyncInfo(
                                on_wait=extra[i:i + max_waits], on_update=[]),
                            bass_nofuse=True,
                        )
                        new_insts.append(nop)
                new_insts.append(inst)
            blk.instructions[:] = new_insts
    return n_split

# ---------------------------------------------------------------------------
# Problem constants (hardcoded; kernel.py must be self-contained)
# ---------------------------------------------------------------------------
N_IMG, C, H, W = 4, 512, 64, 64
KC, VC, OC = 256, 256, 512
L = H * W  # 4096
QH = L // 2  # queries per core
N_CORES = 8
EPS = 1e-5

NCC = C // 128  # 4   c chunks
NKC = KC // 128  # 2  kc chunks
NVC = VC // 128  # 2  vc chunks
NKI = L // 128  # 32  key chunks
NQB = QH // 512  # 4  query blocks per core
NLB = L // 512  # 8   l blocks


def _bcast(ap, p=128):
    """Broadcast a 1-D DRAM AP across p partitions."""
    return bass.AP(tensor=ap.tensor, offset=ap.offset, ap=[[0, p], list(ap.ap[0])])


def _build_program():
    nc = bass.Bass("TRN2", target_bir_lowering=False, debug=False,
                   num_devices=N_CORES)

    xq_ap = nc.dram_tensor("xq", [C, QH], F32R, kind="ExternalInput").ap()
    xo_ap = nc.dram_tensor("xo", [C, QH], F32R, kind="ExternalInput").ap()
    wqT_ap = nc.dram_tensor("wqT", [C, KC], F32R, kind="ExternalInput").ap()
    wkT_ap = nc.dram_tensor("wkT", [C, KC], F32R, kind="ExternalInput").ap()
    wvT_ap = nc.dram_tensor("wvT", [C, VC], F32R, kind="ExternalInput").ap()
    wWT_ap = nc.dram_tensor("wWT", [VC, OC], F32R, kind="ExternalInput").ap()
    bq_ap = nc.dram_tensor("bq", [KC], F32, kind="ExternalInput").ap()
    bk_ap = nc.dram_tensor("bk", [KC], F32, kind="ExternalInput").ap()
    bv_ap = nc.dram_tensor("bv", [VC], F32, kind="ExternalInput").ap()
    bW_ap = nc.dram_tensor("bW", [OC], F32, kind="ExternalInput").ap()
    out_ap = nc.dram_tensor("out_t", [QH, OC], F32, kind="ExternalOutput").ap()

    NQT = QH // 512  # 4 column-quarters per half

    with tile.TileContext(nc) as tc, ExitStack() as stack:
        consts = stack.enter_context(tc.tile_pool(name="consts", bufs=1))
        persist = stack.enter_context(tc.tile_pool(name="persist", bufs=1))
        # one shared pool for ALL transient matmul PSUM outputs (2 banks),
        # accumulators get their own pools: sum+rtp 2, ctx 2, out 2 -> 8 total
        mm_ps = stack.enter_context(tc.tile_pool(name="mm_ps", bufs=3,
                                                 space="PSUM"))
        ctx_psum = stack.enter_context(tc.tile_pool(name="ctx_psum", bufs=1,
                                                    space="PSUM"))
        o_psum = stack.enter_context(tc.tile_pool(name="o_psum", bufs=2,
                                                  space="PSUM"))
        sb_psum = stack.enter_context(tc.tile_pool(name="sb_psum", bufs=1,
                                                   space="PSUM"))
        acc_pool = stack.enter_context(tc.tile_pool(name="acc_sb", bufs=2))
        pt_pool = stack.enter_context(tc.tile_pool(name="pt", bufs=8))
        ctx_pool = stack.enter_context(tc.tile_pool(name="ctx_sb", bufs=2))
        o_pool = stack.enter_context(tc.tile_pool(name="o_sb", bufs=2))
        r_pool = stack.enter_context(tc.tile_pool(name="r_sb", bufs=2))
        dram_pool = stack.enter_context(tc.tile_pool(name="dramp", bufs=2,
                                                     space="DRAM"))
        xo_pool = stack.enter_context(tc.tile_pool(name="xopool", bufs=1))

        # ---- weights / consts ----
        wq_s = consts.tile([128, NCC, KC], F32R, tag="wq")
        nc.sync.dma_start(wq_s[:], wqT_ap.rearrange("(a p) k -> p a k", p=128))
        wk_s = consts.tile([128, NCC, KC], F32R, tag="wk")
        nc.sync.dma_start(wk_s[:], wkT_ap.rearrange("(a p) k -> p a k", p=128))
        wv_s = consts.tile([128, NCC, VC], F32R, tag="wv")
        nc.sync.dma_start(wv_s[:], wvT_ap.rearrange("(a p) k -> p a k", p=128))
        wW_s = consts.tile([128, NVC, OC], F32R, tag="wW")
        nc.sync.dma_start(wW_s[:], wWT_ap.rearrange("(a p) k -> p a k", p=128))
        bq_s = consts.tile([128, NKC], F32, tag="bq")
        nc.sync.dma_start(bq_s[:], bq_ap.rearrange("(a p) -> p a", p=128))
        bk_s = consts.tile([128, NKC], F32, tag="bk")
        nc.sync.dma_start(bk_s[:], bk_ap.rearrange("(a p) -> p a", p=128))
        bv_s = consts.tile([128, VC], F32, tag="bv")
        nc.sync.dma_start(bv_s[:], _bcast(bv_ap))
        bW_s = consts.tile([128, OC], F32, tag="bW")
        nc.sync.dma_start(bW_s[:], _bcast(bW_ap))
        ones_f = consts.tile([128, 1], F32, tag="onesf")
        nc.vector.memset(ones_f[:], 1.0)
        ones_s = consts.tile([128, 1], F32R, tag="ones")
        nc.vector.tensor_copy(ones_s[:], ones_f[:])
        warm_exp = consts.tile([128, 1], F32, tag="wexp")
        ones_mf = consts.tile([128, 128], F32, tag="onesmf")
        nc.vector.memset(ones_mf[:], 1.0)
        ones_m = consts.tile([128, 128], F32R, tag="onesm")
        nc.vector.tensor_copy(ones_m[:], ones_mf[:])
        ident1 = consts.tile([1, 1], F32, tag="id1")
        nc.vector.memset(ident1[:], 1.0)

        # ---- persistent activations ----
        k_s = [persist.tile([128, L], F32R, tag=f"k{j}", name=f"k{j}")
               for j in range(NKC)]
        q_s = [persist.tile([128, QH], F32R, tag=f"q{j}", name=f"q{j}")
               for j in range(NKC)]
        vT_s = persist.tile([128, NKI, VC], F32R, tag="vT")

        def r(ap):
            return ap

        # ---- striped input DMAs (xq first; precise quarter-level deps) ----
        xq_s = [[None] * NQT for _ in range(NCC)]
        xo_s = [[None] * NQT for _ in range(NCC)]

        def stripe(pool, store, src_ap, pfx, t, ci):
            xt = pool.tile([128, 512], F32R, tag=f"{pfx}{ci}_{t}",
                           name=f"{pfx}{ci}_{t}")
            nc.sync.dma_start(
                xt[:], src_ap[ci * 128:(ci + 1) * 128, t * 512:(t + 1) * 512])
            store[ci][t] = xt

        def proj_quarter(xs, b, glob_b):
            # K chunk, (Q chunk if own half), V^T for one 512-column quarter
            for j in range(NKC):
                ps = mm_ps.tile([128, 512], F32, tag="mm", name=f"pk{j}_{glob_b}")
                for ci in range(NCC):
                    nc.tensor.matmul(
                        ps[:],
                        r(wk_s[:, ci, j * 128:(j + 1) * 128]),
                        r(xs[ci][b][:]),
                        start=(ci == 0), stop=(ci == NCC - 1))
                nc.vector.tensor_scalar_add(
                    k_s[j][:, glob_b * 512:(glob_b + 1) * 512], ps[:],
                    bk_s[:, j:j + 1])
            if glob_b < NQT:
                for j in range(NKC):
                    ps = mm_ps.tile([128, 512], F32, tag="mm",
                                    name=f"pq{j}_{glob_b}")
                    for ci in range(NCC):
                        nc.tensor.matmul(
                            ps[:],
                            r(wq_s[:, ci, j * 128:(j + 1) * 128]),
                            r(xs[ci][b][:]),
                            start=(ci == 0), stop=(ci == NCC - 1))
                    nc.vector.tensor_scalar_add(
                        q_s[j][:, glob_b * 512:(glob_b + 1) * 512], ps[:],
                        bq_s[:, j:j + 1])
            for kk in range(4 * b, 4 * b + 4):
                gki = glob_b * 4 + (kk - 4 * b)
                ps = mm_ps.tile([128, VC], F32, tag="mm", name=f"pv{gki}")
                for ci in range(NCC):
                    nc.tensor.matmul(
                        ps[:],
                        r(xs[ci][kk // 4][:, (kk % 4) * 128:(kk % 4 + 1) * 128]),
                        r(wv_s[:, ci, :]),
                        start=(ci == 0), stop=(ci == NCC - 1))
                nc.vector.tensor_add(vT_s[:, gki, :], ps[:], bv_s[:])

        # ---- attention ----
        def attn_qblock(qb, part, state):
            qo = qb * 512
            if part == "prefix":
                state.setdefault("pt", {})
            elif part in ("full", "first", "second") and "acc" not in state:
                state["acc"] = acc_pool.tile([128, 512], F32R, tag="acc",
                                             name=f"acc{qb}")
                state["ctx_ps"] = [
                    ctx_psum.tile([128, 512], F32, tag=f"ctx{j}",
                                  name=f"ctx{qb}_{j}")
                    for j in range(NVC)]
                state.setdefault("pt", {})
            acc = state.get("acc")
            ctx_ps = state.get("ctx_ps")
            pt_tiles = state["pt"]

            def emit_s(ki):
                ps = mm_ps.tile([128, 512], F32, tag="mm", name=f"s{qb}_{ki}")
                for j in range(NKC):
                    nc.tensor.matmul(
                        ps[:],
                        r(k_s[j][:, ki * 128:(ki + 1) * 128]),
                        r(q_s[j][:, qo:qo + 512]),
                        start=(j == 0), stop=(j == NKC - 1))
                pt = pt_pool.tile([128, 512], F32R, tag="pt",
                                  name=f"pt{qb}_{ki}")
                nc.scalar.activation(pt[:], ps[:], ACT.Exp)
                pt_tiles[ki] = pt

            if part == "prefix":
                for ki in range(6):
                    emit_s(ki)
                return

            def emit_acc(ki):
                pt = pt_tiles.pop(ki)
                if ki == 0:
                    nc.vector.tensor_copy(acc[:], pt[:])
                else:
                    nc.vector.tensor_add(acc[:], acc[:], pt[:])
                for j in range(NVC):
                    nc.tensor.matmul(
                        ctx_ps[j][:],
                        r(vT_s[:, ki, j * 128:(j + 1) * 128]),
                        r(pt[:]),
                        start=(ki == 0), stop=(ki == NKI - 1),
                        skip_group_check=True)

            kis = {"full": list(range(NKI)),
                   "first": list(range(NKI // 2)),
                   "second": list(range(NKI // 2, NKI))}[part]
            if kis[0] not in pt_tiles:
                emit_s(kis[0])
            last = kis[-1]
            for ki in kis:
                if ki < last and ki + 1 not in pt_tiles:
                    emit_s(ki + 1)
                emit_acc(ki)
            if part == "first":
                return
            if state.get("next") is not None:
                # pre-emit the next q-block's first S matmuls so the PE has
                # work while DVE copies ctx out of PSUM for this block
                nqb, nstate = state["next"]
                attn_qblock(nqb, "prefix", nstate)

            # softmax denominators: ones^T @ acc -> [1,512], PE row-transpose
            # to [128,4] columns, cheap reciprocal on [128,4]
            sums = sb_psum.tile([1, 512], F32, tag="sbc", name=f"sbc{qb}")
            nc.tensor.matmul(sums[:], r(ones_s[:]), r(acc[:]),
                             start=True, stop=True, skip_group_check=True)
            srow = r_pool.tile([1, 512], F32, tag="srow", name=f"sr{qb}")
            nc.vector.tensor_copy(srow[:], sums[:])
            rtp = mm_ps.tile([128, 4], F32, tag="mm", name=f"rt{qb}")
            for qs in range(4):
                nc.tensor.transpose(rtp[:, qs:qs + 1],
                                    srow[:, qs * 128:(qs + 1) * 128],
                                    ident1[:])
            rcr = r_pool.tile([128, 4], F32, tag="rcr", name=f"rcr{qb}")
            nc.vector.tensor_copy(rcr[:], rtp[:])
            rcol = r_pool.tile([128, 4], F32, tag="rcol", name=f"rc{qb}")
            nc.vector.reciprocal(rcol[:], rcr[:])

            ctx_sb = []
            for j in range(NVC):
                t = ctx_pool.tile([128, 512], F32R, tag=f"ctxs{j}",
                                  name=f"cs{qb}_{j}")
                nc.vector.tensor_copy(t[:], ctx_ps[j][:])
                ctx_sb.append(t)
            for qs in range(4):
                ops = o_psum.tile([128, OC], F32, tag="ops", name=f"o{qb}_{qs}")
                for j in range(NVC):
                    nc.tensor.matmul(
                        ops[:],
                        r(ctx_sb[j][:, qs * 128:(qs + 1) * 128]),
                        r(wW_s[:, j, :]),
                        start=(j == 0), stop=(j == NVC - 1))
                o_sc = o_pool.tile([128, OC], F32, tag="osc", name=f"sc{qb}_{qs}")
                nc.vector.tensor_scalar_mul(o_sc[:], ops[:],
                                            rcol[:, qs:qs + 1])
                o_fin = o_pool.tile([128, OC], F32, tag="ofin",
                                    name=f"of{qb}_{qs}")
                nc.vector.tensor_add(o_fin[:], o_sc[:], bW_s[:])
                nc.sync.dma_start(
                    out_ap[qo + qs * 128: qo + (qs + 1) * 128, :], o_fin[:])

        # ---- program order ----
        with tc.tile_pool(name="xqpool", bufs=1) as xqp:
            for t in range(NQT):
                for ci in range(NCC):
                    stripe(xqp, xq_s, xq_ap, "xq", t, ci)
            for t in range(NQT):
                for ci in range(NCC):
                    stripe(xo_pool, xo_s, xo_ap, "xo", t, ci)

            # PE warm-up on the (tiny, early) weight tiles: release the HAM
            # clock throttle before the projections start
            for wi in range(24):
                wps = mm_ps.tile([1, KC], F32, tag="mm", name=f"warm{wi}")
                nc.tensor.matmul(wps[:], r(ones_s[:]), r(wq_s[:, 0, :]),
                                 start=True, stop=True, skip_group_check=True)

            for b in range(NQT):             # own half: K+Q+V^T per quarter
                proj_quarter(xq_s, b, b)
            # preload the Exp LUT (attention's first exp skips table load)
            nc.scalar.activation(warm_exp[:], ones_f[:], ACT.Exp)

            # qb0 own-half keys run while xo is still streaming in; the
            # other-half projections then fill the PE before qb0 resumes.
            states = [{} for _ in range(NQB)]
            for qb in range(NQB - 1):
                states[qb]["next"] = (qb + 1, states[qb + 1])
            states[NQB - 1]["next"] = None
            attn_qblock(0, "first", states[0])
            for b in range(NQT):
                proj_quarter(xo_s, b, NQT + b)
            attn_qblock(0, "second", states[0])
        for qb in range(1, NQB):
            attn_qblock(qb, "full", states[qb])

    _split_excess_waits(nc)
    return nc


_NC_CACHE = {}


def _get_nc():
    if "nc" not in _NC_CACHE:
        _NC_CACHE["nc"] = _build_program()
    return _NC_CACHE["nc"]


def _prep_in_maps(x, wq, bq, gq, betaq, mq, vq, wk, bk, gk, betak, mk, vk,
                  wv, bv, wW, bW):
    x = np.asarray(x, np.float32)
    invq = np.asarray(gq, np.float32) / np.sqrt(np.asarray(vq, np.float32) + EPS)
    invk = np.asarray(gk, np.float32) / np.sqrt(np.asarray(vk, np.float32) + EPS)
    scale = 1.0 / np.sqrt(np.float32(KC))
    wq_f = (np.asarray(wq, np.float32) * invq[:, None]) * scale
    bq_f = (np.asarray(bq, np.float32) * invq + np.asarray(betaq, np.float32)
            - np.asarray(mq, np.float32) * invq) * scale
    wk_f = np.asarray(wk, np.float32) * invk[:, None]
    bk_f = (np.asarray(bk, np.float32) * invk + np.asarray(betak, np.float32)
            - np.asarray(mk, np.float32) * invk)

    shared = {
        "wqT": np.ascontiguousarray(wq_f.T, np.float32),
        "wkT": np.ascontiguousarray(wk_f.T, np.float32),
        "wvT": np.ascontiguousarray(np.asarray(wv, np.float32).T, np.float32),
        "wWT": np.ascontiguousarray(np.asarray(wW, np.float32).T, np.float32),
        "bq": np.ascontiguousarray(bq_f, np.float32),
        "bk": np.ascontiguousarray(bk_f, np.float32),
        "bv": np.ascontiguousarray(np.asarray(bv, np.float32)),
        "bW": np.ascontiguousarray(np.asarray(bW, np.float32)),
    }
    in_maps = []
    for c in range(N_CORES):
        n, half = c // 2, c % 2
        x_img = x[n].reshape(C, L)
        xq = np.ascontiguousarray(x_img[:, half * QH:(half + 1) * QH])
        xo = np.ascontiguousarray(x_img[:, (1 - half) * QH:(2 - half) * QH])
        in_maps.append({"xq": xq, "xo": xo, **shared})
    return in_maps


def _assemble(results):
    full = np.empty((N_IMG, C if OC == C else OC, L), np.float32)
    for n in range(N_IMG):
        halves = [results[2 * n]["out_t"], results[2 * n + 1]["out_t"]]
        img = np.concatenate(halves, axis=0)  # [L, OC]
        full[n] = img.T
    return full.reshape(N_IMG, OC, H, W)


def run_bass(trace=False, **inputs):
    nc = _get_nc()
    in_maps = _prep_in_maps(**inputs)
    res = run_bass_kernel_spmd(nc, in_maps, core_ids=list(range(N_CORES)),
                               trace=trace)
    return _assemble(res.results), res


def kernel(**inputs):
    out, _ = run_bass(trace=False, **inputs)
    return out


# revision 32
# speedup vs baseline: 1.1198x; 1.0052x over previous
"""Trainium2 Bass kernel for BasicNonLocalBlock (N=4, C=512, H=W=64, KC=VC=256, OC=512).

Sharding: 8 cores = 4 images x 2 query-halves. Each core holds one full image
(for K/V) and computes attention output for 2048 of the 4096 queries.

Per-core dataflow (all matmuls float32r, free dim 512 or 256):
  K  [256,4096] = fold(Wk,BN) @ x            (kc on partitions)
  Q  [256,2048] = fold(Wq,BN,1/sqrt(KC)) @ x_qhalf
  V^T[4096,256] = x^T @ WvT + bv             (kpos on partitions; no transposes)
  per 512-query block, streamed over 32 key chunks of 128:
    S^T[128,512] = K_chunk^T Q_block   (PSUM)
    P^T = exp(S^T)                     (scalar engine, PSUM->SBUF)
    sums[1,512]  += ones^T P^T         (PE accumulation)
    ctx[vc,512]  += V^T_chunk^T P^T    (PE accumulation, vc in 2 chunks)
  r = 1/sums  (transposed to [128,4] via DRAM bounce)
  out^T[q,oc] = ctx^T @ WWT, scaled by r (per-partition) + bW broadcast
Host folds BN + attention scale into weights, transposes/assembles output.
"""

import sys
import types
from contextlib import ExitStack

import numpy as np

# ---------------------------------------------------------------------------
# Environment shims (axon image lacks antenv.axon_hooks; walrus rejects >2
# sync waits on the tail Drain emitted by TileContext).
# ---------------------------------------------------------------------------


def _install_ntff_hook_shim():
    try:
        import antenv
    except ImportError:
        return
    if "antenv.axon_hooks" in sys.modules:
        return
    mod = types.ModuleType("antenv.axon_hooks")
    mod._hook = None

    def set_axon_ntff_profile_hook(h):
        mod._hook = h

    def get_axon_ntff_profile_hook():
        return mod._hook

    mod.set_axon_ntff_profile_hook = set_axon_ntff_profile_hook
    mod.get_axon_ntff_profile_hook = get_axon_ntff_profile_hook
    sys.modules["antenv.axon_hooks"] = mod
    antenv.axon_hooks = mod
    try:
        if "/root/.axon_site" not in sys.path:
            sys.path.insert(0, "/root/.axon_site")
        from trn_agent_boot.trn_boot import _ntff_profile_via_ctypes

        hook = _ntff_profile_via_ctypes("/opt/axon/libaxon_pjrt.so")
        if hook is not None:
            mod._hook = hook
    except Exception:
        pass


_install_ntff_hook_shim()

import concourse.bass as bass
import concourse.bass_isa as bass_isa
import concourse.tile as tile
from concourse import mybir
from concourse.bass_utils import run_bass_kernel_spmd
from concourse.vector_clock import ScopedClock

F32 = mybir.dt.float32
F32R = mybir.dt.float32r
ACT = mybir.ActivationFunctionType


def _patched_drain_and_barrier(self, tick_clock, wait_clock):
    nc = self.nc
    probe = nc.sync.nop(nofuse=True, hint="drain_waits_probe")
    wait_clock.add_sem_waits(probe.ins, ScopedClock({None: tick_clock.global_clock}))
    si = probe.ins.sync_info
    waits = list(si.on_wait or []) if si is not None else []
    if si is not None:
        si.on_wait = waits[:1]
    for w in waits[1:]:
        n = nc.sync.nop(nofuse=True, hint="drain_waits_extra")
        n.ins.sync_info = mybir.SyncInfo(on_wait=[w], on_update=[])
    nc.sync.drain()
    nc.all_engine_barrier()
    assert self.sems is not None
    popped = nc._tile_sem_poison_stack.pop()
    assert popped is self._sem_poison
    nc.clear_and_free_semaphores(list(self.sems.allocated().values()))


tile.TileContext._drain_and_barrier = _patched_drain_and_barrier


def _split_excess_waits(nc):
    """Walrus CoreV3 codegen limits embedded sync waits per instruction
    (1 for self-loading Matmult's LDWEIGHTS struct, 2 elsewhere). Move the
    excess onto same-engine NOPs inserted just before."""
    n_split = 0
    for fn in nc.m.functions:
        for blk in fn.blocks:
            new_insts = []
            for inst in blk.instructions:
                max_waits = 1
                si = getattr(inst, "sync_info", None)
                if si is not None and si.on_wait and len(si.on_wait) > max_waits:
                    waits = list(si.on_wait)
                    extra = waits[:-max_waits]
                    si.on_wait = waits[-max_waits:]
                    for i in range(0, len(extra), max_waits):
                        n_split += 1
                        nop = mybir.InstNoOp(
                            name=f"{inst.name}-ws{i}",
                            engine=inst.engine,
                            ins=[], outs=[],
                            sync_info=mybir.SyncInfo(
                                on_wait=extra[i:i + max_waits], on_update=[]),
                            bass_nofuse=True,
                        )
                        new_insts.append(nop)
                new_insts.append(inst)
            blk.instructions[:] = new_insts
    return n_split

# ---------------------------------------------------------------------------
# Problem constants (hardcoded; kernel.py must be self-contained)
# ---------------------------------------------------------------------------
N_IMG, C, H, W = 4, 512, 64, 64
KC, VC, OC = 256, 256, 512
L = H * W  # 4096
QH = L // 2  # queries per core
N_CORES = 8
EPS = 1e-5

NCC = C // 128  # 4   c chunks
NKC = KC // 128  # 2  kc chunks
NVC = VC // 128  # 2  vc chunks
NKI = L // 128  # 32  key chunks
NQB = QH // 512  # 4  query blocks per core
NLB = L // 512  # 8   l blocks


def _bcast(ap, p=128):
    """Broadcast a 1-D DRAM AP across p partitions."""
    return bass.AP(tensor=ap.tensor, offset=ap.offset, ap=[[0, p], list(ap.ap[0])])


def _build_program():
    nc = bass.Bass("TRN2", target_bir_lowering=False, debug=False,
                   num_devices=N_CORES)

    xq_ap = nc.dram_tensor("xq", [C, QH], F32R, kind="ExternalInput").ap()
    xo_ap = nc.dram_tensor("xo", [C, QH], F32R, kind="ExternalInput").ap()
    wqT_ap = nc.dram_tensor("wqT", [C, KC], F32R, kind="ExternalInput").ap()
    wkT_ap = nc.dram_tensor("wkT", [C, KC], F32R, kind="ExternalInput").ap()
    wvT_ap = nc.dram_tensor("wvT", [C, VC], F32R, kind="ExternalInput").ap()
    wWT_ap = nc.dram_tensor("wWT", [VC, OC], F32R, kind="ExternalInput").ap()
    bq_ap = nc.dram_tensor("bq", [KC], F32, kind="ExternalInput").ap()
    bk_ap = nc.dram_tensor("bk", [KC], F32, kind="ExternalInput").ap()
    bv_ap = nc.dram_tensor("bv", [VC], F32, kind="ExternalInput").ap()
    bW_ap = nc.dram_tensor("bW", [OC], F32, kind="ExternalInput").ap()
    out_ap = nc.dram_tensor("out_t", [QH, OC], F32, kind="ExternalOutput").ap()

    NQT = QH // 512  # 4 column-quarters per half

    with tile.TileContext(nc) as tc, ExitStack() as stack:
        consts = stack.enter_context(tc.tile_pool(name="consts", bufs=1))
        persist = stack.enter_context(tc.tile_pool(name="persist", bufs=1))
        # one shared pool for ALL transient matmul PSUM outputs (2 banks),
        # accumulators get their own pools: sum+rtp 2, ctx 2, out 2 -> 8 total
        mm_ps = stack.enter_context(tc.tile_pool(name="mm_ps", bufs=3,
                                                 space="PSUM"))
        ctx_psum = stack.enter_context(tc.tile_pool(name="ctx_psum", bufs=1,
                                                    space="PSUM"))
        o_psum = stack.enter_context(tc.tile_pool(name="o_psum", bufs=2,
                                                  space="PSUM"))
        sb_psum = stack.enter_context(tc.tile_pool(name="sb_psum", bufs=1,
                                                   space="PSUM"))
        acc_pool = stack.enter_context(tc.tile_pool(name="acc_sb", bufs=2))
        pt_pool = stack.enter_context(tc.tile_pool(name="pt", bufs=8))
        ctx_pool = stack.enter_context(tc.tile_pool(name="ctx_sb", bufs=2))
        o_pool = stack.enter_context(tc.tile_pool(name="o_sb", bufs=2))
        r_pool = stack.enter_context(tc.tile_pool(name="r_sb", bufs=2))
        dram_pool = stack.enter_context(tc.tile_pool(name="dramp", bufs=2,
                                                     space="DRAM"))
        xo_pool = stack.enter_context(tc.tile_pool(name="xopool", bufs=1))

        # ---- weights / consts ----
        wq_s = consts.tile([128, NCC, KC], F32R, tag="wq")
        nc.sync.dma_start(wq_s[:], wqT_ap.rearrange("(a p) k -> p a k", p=128))
        wk_s = consts.tile([128, NCC, KC], F32R, tag="wk")
        nc.sync.dma_start(wk_s[:], wkT_ap.rearrange("(a p) k -> p a k", p=128))
        wv_s = consts.tile([128, NCC, VC], F32R, tag="wv")
        nc.sync.dma_start(wv_s[:], wvT_ap.rearrange("(a p) k -> p a k", p=128))
        wW_s = consts.tile([128, NVC, OC], F32R, tag="wW")
        nc.sync.dma_start(wW_s[:], wWT_ap.rearrange("(a p) k -> p a k", p=128))
        bq_s = consts.tile([128, NKC], F32, tag="bq")
        nc.sync.dma_start(bq_s[:], bq_ap.rearrange("(a p) -> p a", p=128))
        bk_s = consts.tile([128, NKC], F32, tag="bk")
        nc.sync.dma_start(bk_s[:], bk_ap.rearrange("(a p) -> p a", p=128))
        bv_s = consts.tile([128, VC], F32, tag="bv")
        nc.sync.dma_start(bv_s[:], _bcast(bv_ap))
        bW_s = consts.tile([128, OC], F32, tag="bW")
        nc.sync.dma_start(bW_s[:], _bcast(bW_ap))
        ones_f = consts.tile([128, 1], F32, tag="onesf")
        nc.vector.memset(ones_f[:], 1.0)
        ones_s = consts.tile([128, 1], F32R, tag="ones")
        nc.vector.tensor_copy(ones_s[:], ones_f[:])
        warm_exp = consts.tile([128, 1], F32, tag="wexp")
        ones_mf = consts.tile([128, 128], F32, tag="onesmf")
        nc.vector.memset(ones_mf[:], 1.0)
        ones_m = consts.tile([128, 128], F32R, tag="onesm")
        nc.vector.tensor_copy(ones_m[:], ones_mf[:])
        ident1 = consts.tile([1, 1], F32, tag="id1")
        nc.vector.memset(ident1[:], 1.0)

        # ---- persistent activations ----
        k_s = [persist.tile([128, L], F32R, tag=f"k{j}", name=f"k{j}")
               for j in range(NKC)]
        q_s = [persist.tile([128, QH], F32R, tag=f"q{j}", name=f"q{j}")
               for j in range(NKC)]
        vT_s = persist.tile([128, NKI, VC], F32R, tag="vT")

        def r(ap):
            return ap

        # ---- striped input DMAs (xq first; precise quarter-level deps) ----
        xq_s = [[None] * NQT for _ in range(NCC)]
        xo_s = [[None] * NQT for _ in range(NCC)]

        def stripe(pool, store, src_ap, pfx, t, ci):
            xt = pool.tile([128, 512], F32R, tag=f"{pfx}{ci}_{t}",
                           name=f"{pfx}{ci}_{t}")
            nc.sync.dma_start(
                xt[:], src_ap[ci * 128:(ci + 1) * 128, t * 512:(t + 1) * 512])
            store[ci][t] = xt

        def proj_quarter(xs, b, glob_b):
            # K chunk, (Q chunk if own half), V^T for one 512-column quarter
            for j in range(NKC):
                ps = mm_ps.tile([128, 512], F32, tag="mm", name=f"pk{j}_{glob_b}")
                for ci in range(NCC):
                    nc.tensor.matmul(
                        ps[:],
                        r(wk_s[:, ci, j * 128:(j + 1) * 128]),
                        r(xs[ci][b][:]),
                        start=(ci == 0), stop=(ci == NCC - 1))
                nc.vector.tensor_scalar_add(
                    k_s[j][:, glob_b * 512:(glob_b + 1) * 512], ps[:],
                    bk_s[:, j:j + 1])
            if glob_b < NQT:
                for j in range(NKC):
                    ps = mm_ps.tile([128, 512], F32, tag="mm",
                                    name=f"pq{j}_{glob_b}")
                    for ci in range(NCC):
                        nc.tensor.matmul(
                            ps[:],
                            r(wq_s[:, ci, j * 128:(j + 1) * 128]),
                            r(xs[ci][b][:]),
                            start=(ci == 0), stop=(ci == NCC - 1))
                    nc.vector.tensor_scalar_add(
                        q_s[j][:, glob_b * 512:(glob_b + 1) * 512], ps[:],
                        bq_s[:, j:j + 1])
            for kk in range(4 * b, 4 * b + 4):
                gki = glob_b * 4 + (kk - 4 * b)
                ps = mm_ps.tile([128, VC], F32, tag="mm", name=f"pv{gki}")
                for ci in range(NCC):
                    nc.tensor.matmul(
                        ps[:],
                        r(xs[ci][kk // 4][:, (kk % 4) * 128:(kk % 4 + 1) * 128]),
                        r(wv_s[:, ci, :]),
                        start=(ci == 0), stop=(ci == NCC - 1))
                nc.vector.tensor_add(vT_s[:, gki, :], ps[:], bv_s[:])

        # ---- attention ----
        def attn_qblock(qb, part, state):
            qo = qb * 512
            if part == "prefix":
                state.setdefault("pt", {})
            elif part in ("full", "first", "second") and "acc" not in state:
                state["acc"] = acc_pool.tile([128, 512], F32R, tag="acc",
                                             name=f"acc{qb}")
                state["ctx_ps"] = [
                    ctx_psum.tile([128, 512], F32, tag=f"ctx{j}",
                                  name=f"ctx{qb}_{j}")
                    for j in range(NVC)]
                state.setdefault("pt", {})
            acc = state.get("acc")
            ctx_ps = state.get("ctx_ps")
            pt_tiles = state["pt"]

            def emit_s(ki):
                ps = mm_ps.tile([128, 512], F32, tag="mm", name=f"s{qb}_{ki}")
                for j in range(NKC):
                    nc.tensor.matmul(
                        ps[:],
                        r(k_s[j][:, ki * 128:(ki + 1) * 128]),
                        r(q_s[j][:, qo:qo + 512]),
                        start=(j == 0), stop=(j == NKC - 1))
                pt = pt_pool.tile([128, 512], F32R, tag="pt",
                                  name=f"pt{qb}_{ki}")
                nc.scalar.activation(pt[:], ps[:], ACT.Exp)
                pt_tiles[ki] = pt

            if part == "prefix":
                for ki in range(8):
                    emit_s(ki)
                return

            def emit_acc(ki):
                pt = pt_tiles.pop(ki)
                if ki == 0:
                    nc.vector.tensor_copy(acc[:], pt[:])
                else:
                    nc.vector.tensor_add(acc[:], acc[:], pt[:])
                for j in range(NVC):
                    nc.tensor.matmul(
                        ctx_ps[j][:],
                        r(vT_s[:, ki, j * 128:(j + 1) * 128]),
                        r(pt[:]),
                        start=(ki == 0), stop=(ki == NKI - 1),
                        skip_group_check=True)

            kis = {"full": list(range(NKI)),
                   "first": list(range(NKI // 2)),
                   "second": list(range(NKI // 2, NKI))}[part]
            if kis[0] not in pt_tiles:
                emit_s(kis[0])
            last = kis[-1]
            for ki in kis:
                if ki < last and ki + 1 not in pt_tiles:
                    emit_s(ki + 1)
                emit_acc(ki)
            if part == "first":
                return
            if state.get("next") is not None:
                # pre-emit the next q-block's first S matmuls so the PE has
                # work while DVE copies ctx out of PSUM for this block
                nqb, nstate = state["next"]
                attn_qblock(nqb, "prefix", nstate)

            # softmax denominators: ones^T @ acc -> [1,512], PE row-transpose
            # to [128,4] columns, cheap reciprocal on [128,4]
            sums = sb_psum.tile([1, 512], F32, tag="sbc", name=f"sbc{qb}")
            nc.tensor.matmul(sums[:], r(ones_s[:]), r(acc[:]),
                             start=True, stop=True, skip_group_check=True)
            srow = r_pool.tile([1, 512], F32, tag="srow", name=f"sr{qb}")
            nc.vector.tensor_copy(srow[:], sums[:])
            rtp = mm_ps.tile([128, 4], F32, tag="mm", name=f"rt{qb}")
            for qs in range(4):
                nc.tensor.transpose(rtp[:, qs:qs + 1],
                                    srow[:, qs * 128:(qs + 1) * 128],
                                    ident1[:])
            rcr = r_pool.tile([128, 4], F32, tag="rcr", name=f"rcr{qb}")
            nc.vector.tensor_copy(rcr[:], rtp[:])
            rcol = r_pool.tile([128, 4], F32, tag="rcol", name=f"rc{qb}")
            nc.vector.reciprocal(rcol[:], rcr[:])

            ctx_sb = []
            for j in range(NVC):
                t = ctx_pool.tile([128, 512], F32R, tag=f"ctxs{j}",
                                  name=f"cs{qb}_{j}")
                nc.vector.tensor_copy(t[:], ctx_ps[j][:])
                ctx_sb.append(t)
            for qs in range(4):
                ops = o_psum.tile([128, OC], F32, tag="ops", name=f"o{qb}_{qs}")
                for j in range(NVC):
                    nc.tensor.matmul(
                        ops[:],
                        r(ctx_sb[j][:, qs * 128:(qs + 1) * 128]),
                        r(wW_s[:, j, :]),
                        start=(j == 0), stop=(j == NVC - 1))
                o_sc = o_pool.tile([128, OC], F32, tag="osc", name=f"sc{qb}_{qs}")
                nc.vector.tensor_scalar_mul(o_sc[:], ops[:],
                                            rcol[:, qs:qs + 1])
                o_fin = o_pool.tile([128, OC], F32, tag="ofin",
                                    name=f"of{qb}_{qs}")
                nc.vector.tensor_add(o_fin[:], o_sc[:], bW_s[:])
                nc.sync.dma_start(
                    out_ap[qo + qs * 128: qo + (qs + 1) * 128, :], o_fin[:])

        # ---- program order ----
        with tc.tile_pool(name="xqpool", bufs=1) as xqp:
            for t in range(NQT):
                for ci in range(NCC):
                    stripe(xqp, xq_s, xq_ap, "xq", t, ci)
            for t in range(NQT):
                for ci in range(NCC):
                    stripe(xo_pool, xo_s, xo_ap, "xo", t, ci)

            # PE warm-up on the (tiny, early) weight tiles: release the HAM
            # clock throttle before the projections start
            for wi in range(24):
                wps = mm_ps.tile([1, KC], F32, tag="mm", name=f"warm{wi}")
                nc.tensor.matmul(wps[:], r(ones_s[:]), r(wq_s[:, 0, :]),
                                 start=True, stop=True, skip_group_check=True)

            for b in range(NQT):             # own half: K+Q+V^T per quarter
                proj_quarter(xq_s, b, b)
            # preload the Exp LUT (attention's first exp skips table load)
            nc.scalar.activation(warm_exp[:], ones_f[:], ACT.Exp)

            # qb0 own-half keys run while xo is still streaming in; the
            # other-half projections then fill the PE before qb0 resumes.
            states = [{} for _ in range(NQB)]
            for qb in range(NQB - 1):
                states[qb]["next"] = (qb + 1, states[qb + 1])
            states[NQB - 1]["next"] = None
            attn_qblock(0, "first", states[0])
            for b in range(NQT):
                proj_quarter(xo_s, b, NQT + b)
            attn_qblock(0, "second", states[0])
        for qb in range(1, NQB):
            attn_qblock(qb, "full", states[qb])

    _split_excess_waits(nc)
    return nc


_NC_CACHE = {}


def _get_nc():
    if "nc" not in _NC_CACHE:
        _NC_CACHE["nc"] = _build_program()
    return _NC_CACHE["nc"]


def _prep_in_maps(x, wq, bq, gq, betaq, mq, vq, wk, bk, gk, betak, mk, vk,
                  wv, bv, wW, bW):
    x = np.asarray(x, np.float32)
    invq = np.asarray(gq, np.float32) / np.sqrt(np.asarray(vq, np.float32) + EPS)
    invk = np.asarray(gk, np.float32) / np.sqrt(np.asarray(vk, np.float32) + EPS)
    scale = 1.0 / np.sqrt(np.float32(KC))
    wq_f = (np.asarray(wq, np.float32) * invq[:, None]) * scale
    bq_f = (np.asarray(bq, np.float32) * invq + np.asarray(betaq, np.float32)
            - np.asarray(mq, np.float32) * invq) * scale
    wk_f = np.asarray(wk, np.float32) * invk[:, None]
    bk_f = (np.asarray(bk, np.float32) * invk + np.asarray(betak, np.float32)
            - np.asarray(mk, np.float32) * invk)

    shared = {
        "wqT": np.ascontiguousarray(wq_f.T, np.float32),
        "wkT": np.ascontiguousarray(wk_f.T, np.float32),
        "wvT": np.ascontiguousarray(np.asarray(wv, np.float32).T, np.float32),
        "wWT": np.ascontiguousarray(np.asarray(wW, np.float32).T, np.float32),
        "bq": np.ascontiguousarray(bq_f, np.float32),
        "bk": np.ascontiguousarray(bk_f, np.float32),
        "bv": np.ascontiguousarray(np.asarray(bv, np.float32)),
        "bW": np.ascontiguousarray(np.asarray(bW, np.float32)),
    }
    in_maps = []
    for c in range(N_CORES):
        n, half = c // 2, c % 2
        x_img = x[n].reshape(C, L)
        xq = np.ascontiguousarray(x_img[:, half * QH:(half + 1) * QH])
        xo = np.ascontiguousarray(x_img[:, (1 - half) * QH:(2 - half) * QH])
        in_maps.append({"xq": xq, "xo": xo, **shared})
    return in_maps


def _assemble(results):
    full = np.empty((N_IMG, C if OC == C else OC, L), np.float32)
    for n in range(N_IMG):
        halves = [results[2 * n]["out_t"], results[2 * n + 1]["out_t"]]
        img = np.concatenate(halves, axis=0)  # [L, OC]
        full[n] = img.T
    return full.reshape(N_IMG, OC, H, W)


def run_bass(trace=False, **inputs):
    nc = _get_nc()
    in_maps = _prep_in_maps(**inputs)
    res = run_bass_kernel_spmd(nc, in_maps, core_ids=list(range(N_CORES)),
                               trace=trace)
    return _assemble(res.results), res


def kernel(**inputs):
    out, _ = run_bass(trace=False, **inputs)
    return out
